# revision 20
# baseline (speedup 1.0000x reference)
"""Trainium2 Bass kernel for nn_CrossAttention (dense transformer block).

Sharding: data-parallel over batch — 8 batch elements, one per NeuronCore.
Each core runs the full block for its batch element:
  bias = Conv1x1(gelu(Conv1x1(log(attn_map[1:,1:] + eps))))
  MHA(q, kv) with bias added to scores; residual + LN; FFN; residual + LN.

v2: bf16 weights/activations on the matmul path, conv-bias pipeline with
batched scalar ops (log+exp share one ACT table set), head-pair packed
attention with DMA-xbar transposes, epilogues on vector/gpsimd.

Self-contained: hardcodes all shapes; host-side numpy prepares transposed /
packed weight layouts per core.
"""

import numpy as np
import ml_dtypes

import concourse.bass as bass
import concourse.mybir as mybir
import concourse.tile as tile
from concourse import bacc
from concourse.bass import ts
from concourse.bass_utils import run_bass_kernel_spmd
from concourse.masks import make_identity

AF = mybir.ActivationFunctionType
ALU = mybir.AluOpType
AX = mybir.AxisListType

B, S, D, H, DH, FF = 8, 512, 1024, 16, 64, 4096
CH, CHID = 16, 32
EPS_LOG = 1e-6
EPS_LN = 1e-6
P = 128
NQT = S // P          # 4 q-tiles
ND = D // P           # 8 d-blocks
NFF = FF // P         # 32 ff-blocks
AM = 513              # attn_map edge
NQI = 8               # q rows per conv group (8 groups of 16 partitions/hemi)

fp32 = mybir.dt.float32
bf16 = mybir.dt.bfloat16
fp8 = mybir.dt.float8e4
DR = mybir.MatmulPerfMode.DoubleRow

# fp8 e4m3 for both FFN matmuls (DoubleRow, 2 MACs/cell/cycle).
# Measured: pushes rel err to 2.8e-2 (> 2e-2 gate) — e4m3's 3 mantissa
# bits give ~2-3% rms weight error. Keep bf16.
FFN_FP8 = False

# 'dma' = xbar DMA transpose for attention probs, 'pe' = tensor-engine
ATT_TRANSPOSE = "dma"

_CACHED = {}


def _layernorm(nc, pool, out_ap, x_ap, gb, bb, eps_c):
    """out = (x - mean(x)) * rsqrt(var(x) + eps) * g + b over free dim (D)."""
    nsub = D // 512
    stats = pool.tile([P, nsub, nc.vector.BN_STATS_DIM], fp32, tag="ln_stats")
    for i in range(nsub):
        nc.vector.bn_stats(out=stats[:, i, :], in_=x_ap[:, ts(i, 512)])
    mv = pool.tile([P, nc.vector.BN_AGGR_DIM], fp32, tag="ln_mv")
    nc.vector.bn_aggr(out=mv, in_=stats)
    rstd = pool.tile([P, 1], fp32, tag="ln_rstd")
    nc.scalar.activation(rstd, mv[:, 1:2], AF.Sqrt, bias=eps_c, scale=1.0)
    nc.vector.reciprocal(out=rstd, in_=rstd)
    u = pool.tile([P, D], fp32, tag="ln_u")
    nc.vector.scalar_tensor_tensor(
        out=u, in0=x_ap, scalar=mv[:, 0:1], in1=gb,
        op0=ALU.subtract, op1=ALU.mult,
    )
    nc.vector.scalar_tensor_tensor(
        out=out_ap, in0=u, scalar=rstd[:, 0:1], in1=bb,
        op0=ALU.mult, op1=ALU.add,
    )


def _attention_qt(nc, qt, biasq, QtT, KtT, Vsb, ctxT, ident_b, p3sb, p3ps):
    """Attention for one 128-row q-tile, bias already staged in biasq."""
    den = p3sb.tile([P, H], fp32, tag="den")
    rec = p3sb.tile([P, H], fp32, tag="rec")
    for hp in range(8):  # 8 head pairs
        sc2 = p3ps.tile([P, 2, S], fp32, tag="sc2")
        for j in range(2):
            h = hp * 2 + j
            hb, ho = (h * DH) // P, (h * DH) % P
            nc.tensor.matmul(
                sc2[:, j, :],
                QtT[ho : ho + DH, hb, ts(qt, P)],
                KtT[ho : ho + DH, hb, :],
                start=True, stop=False,
            )
        for j in range(2):
            h = hp * 2 + j
            nc.tensor.matmul(
                sc2[:, j, :], ident_b, biasq[:, h, :],
                start=False, stop=True,
            )
        att2 = p3sb.tile([P, 2, S], bf16, tag="att2")
        nc.scalar.activation(
            att2.rearrange("p a b -> p (a b)"),
            sc2.rearrange("p a b -> p (a b)"),
            AF.Exp,
        )
        nc.vector.tensor_reduce(
            out=den[:, hp * 2 : hp * 2 + 2], in_=att2,
            axis=AX.X, op=ALU.add,
        )
        nc.vector.reciprocal(
            out=rec[:, hp * 2 : hp * 2 + 2],
            in_=den[:, hp * 2 : hp * 2 + 2],
        )
        cx = p3ps.tile([P, P], fp32, tag="cx")
        for j in range(2):
            h = hp * 2 + j
            attn = p3sb.tile([P, S], bf16, tag="attn", bufs=3)
            nc.vector.tensor_scalar_mul(
                attn, att2[:, j, :], rec[:, h : h + 1]
            )
            atTs = p3sb.tile([P, NQT, P], bf16, tag="atTs", bufs=3)
            if ATT_TRANSPOSE == "dma":
                nc.sync.dma_start_transpose(atTs, attn)
            else:
                atT_ps = p3ps.tile([P, S], bf16, tag="atT")
                for kt in range(NQT):
                    nc.tensor.transpose(
                        atT_ps[:, ts(kt, P)], attn[:, ts(kt, P)], ident_b
                    )
                nc.vector.tensor_copy(
                    atTs.rearrange("p a b -> p (a b)"), atT_ps
                )
            for kt in range(NQT):
                nc.tensor.matmul(
                    cx[j * DH : (j + 1) * DH, :],
                    Vsb[:, kt, h * DH : (h + 1) * DH],
                    atTs[:, kt, :],
                    start=(kt == 0), stop=(kt == NQT - 1),
                    tile_position=(0, j * DH),
                )
        nc.vector.tensor_copy(ctxT[:, hp, ts(qt, P)], cx)


def build_program():
    nc = bacc.Bacc(None)

    # ---------------- DRAM I/O ----------------
    qT_e = nc.dram_tensor("qT", [D, S], bf16, kind="ExternalInput")
    kvT_e = nc.dram_tensor("kvT", [D, S], bf16, kind="ExternalInput")
    qin_e = nc.dram_tensor("qin", [S, D], fp32, kind="ExternalInput")
    amap_e = nc.dram_tensor("amap", [CH, AM, AM], fp32, kind="ExternalInput")
    wqT_e = nc.dram_tensor("wqT", [D, D], bf16, kind="ExternalInput")
    wkT_e = nc.dram_tensor("wkT", [D, D], bf16, kind="ExternalInput")
    wvT_e = nc.dram_tensor("wvT", [D, D], bf16, kind="ExternalInput")
    wmT_e = nc.dram_tensor("wmT", [D, D], bf16, kind="ExternalInput")
    fdt = fp8 if FFN_FP8 else bf16
    # FFN1 weights: [ffb, p, (dpair b, j, f)] = Wf1.T[(2b+j)*128+p, ffb*128+f]
    wf1P_e = nc.dram_tensor("wf1P", [NFF, P, D], fdt, kind="ExternalInput")
    # FFN2 weights: [fb, p, (j, d)] = Wf2.T[(2fb+j)*128+p, d]
    wf2P_e = nc.dram_tensor("wf2P", [NFF // 2, P, 2 * D], fdt, kind="ExternalInput")
    c1A_e = nc.dram_tensor("c1A", [P, P], bf16, kind="ExternalInput")
    c1B_e = nc.dram_tensor("c1B", [P, P], bf16, kind="ExternalInput")
    c2A_e = nc.dram_tensor("c2A", [P, P], bf16, kind="ExternalInput")
    c2B_e = nc.dram_tensor("c2B", [P, P], bf16, kind="ExternalInput")
    # per-partition bias columns
    bqc_e = nc.dram_tensor("bqc", [P, ND], fp32, kind="ExternalInput")   # bq/8
    bkc_e = nc.dram_tensor("bkc", [P, ND], fp32, kind="ExternalInput")
    bc1Ar_e = nc.dram_tensor("bc1Ar", [1, P], bf16, kind="ExternalInput")
    bc1Br_e = nc.dram_tensor("bc1Br", [1, P], bf16, kind="ExternalInput")
    ones5r_e = nc.dram_tensor("ones5r", [1, S], bf16, kind="ExternalInput")
    bc2c_e = nc.dram_tensor("bc2c", [P, 1], fp32, kind="ExternalInput")
    bf1c_e = nc.dram_tensor("bf1c", [P, NFF], fp32, kind="ExternalInput")
    # bias rows (K=1 matmul trick)
    bvr_e = nc.dram_tensor("bvr", [1, D], bf16, kind="ExternalInput")
    bmr_e = nc.dram_tensor("bmr", [1, D], bf16, kind="ExternalInput")
    bf2r_e = nc.dram_tensor("bf2r", [1, D], bf16, kind="ExternalInput")
    onesb_e = nc.dram_tensor("onesb", [1, P], bf16, kind="ExternalInput")
    # LN params as rows
    g1r_e = nc.dram_tensor("g1r", [1, D], fp32, kind="ExternalInput")
    b1r_e = nc.dram_tensor("b1r", [1, D], fp32, kind="ExternalInput")
    g2r_e = nc.dram_tensor("g2r", [1, D], fp32, kind="ExternalInput")
    b2r_e = nc.dram_tensor("b2r", [1, D], fp32, kind="ExternalInput")

    out_e = nc.dram_tensor("out", [S, D], fp32, kind="ExternalOutput")

    with tile.TileContext(nc) as tc:
        # ------------- persistent pools -------------
        const_cm = tc.tile_pool(name="const", bufs=1)
        const = const_cm.__enter__()
        dram_cm = tc.tile_pool(name="dstage", bufs=1, space="DRAM")
        dram = dram_cm.__enter__()
        bstage = dram.tile([S, H, S], bf16)
        resid_cm = tc.tile_pool(name="resid", bufs=1)  # ctxT/xln (ph3-7)
        resid = resid_cm.__enter__()
        bigE_cm = tc.tile_pool(name="bigE", bufs=1)   # Qt/Kt/V (ph1-3)
        bigE = bigE_cm.__enter__()

        ident_b = const.tile([P, P], bf16)
        make_identity(nc, ident_b)
        ident_f = const.tile([P, P], fp32)
        make_identity(nc, ident_f)

        eps_log_c = const.tile([P, 1], fp32)
        nc.vector.memset(eps_log_c, EPS_LOG)
        eps_ln_c = const.tile([P, 1], fp32)
        nc.vector.memset(eps_ln_c, EPS_LN)

        c1A = const.tile([P, P], bf16)
        c1B = const.tile([P, P], bf16)
        c2A = const.tile([P, P], bf16)
        c2B = const.tile([P, P], bf16)
        nc.sync.dma_start(out=c1A, in_=c1A_e[:, :])
        nc.sync.dma_start(out=c1B, in_=c1B_e[:, :])
        nc.sync.dma_start(out=c2A, in_=c2A_e[:, :])
        nc.sync.dma_start(out=c2B, in_=c2B_e[:, :])
        bc1Ar = const.tile([1, P], bf16)
        bc1Br = const.tile([1, P], bf16)
        ones5r = const.tile([1, S], bf16)
        bc2c = const.tile([P, 1], fp32)
        nc.sync.dma_start(out=bc1Ar, in_=bc1Ar_e[:, :])
        nc.sync.dma_start(out=bc1Br, in_=bc1Br_e[:, :])
        nc.sync.dma_start(out=ones5r, in_=ones5r_e[:, :])
        nc.sync.dma_start(out=bc2c, in_=bc2c_e[:, :])
        bqc = const.tile([P, ND], fp32)
        bkc = const.tile([P, ND], fp32)
        bf1c = const.tile([P, NFF], fp32)
        nc.sync.dma_start(out=bqc, in_=bqc_e[:, :])
        nc.sync.dma_start(out=bkc, in_=bkc_e[:, :])
        nc.sync.dma_start(out=bf1c, in_=bf1c_e[:, :])
        bvr = const.tile([1, D], bf16)
        bmr = const.tile([1, D], bf16)
        bf2r = const.tile([1, D], bf16)
        onesb = const.tile([1, P], bf16)
        nc.sync.dma_start(out=bvr, in_=bvr_e[:, :])
        nc.sync.dma_start(out=bmr, in_=bmr_e[:, :])
        nc.sync.dma_start(out=bf2r, in_=bf2r_e[:, :])
        nc.sync.dma_start(out=onesb, in_=onesb_e[:, :])

        # LN param broadcast tiles [128, D]
        g1b = const.tile([P, D], fp32)
        b1b = const.tile([P, D], fp32)
        g2b = const.tile([P, D], fp32)
        b2b = const.tile([P, D], fp32)
        for dst, src_e in ((g1b, g1r_e), (b1b, b1r_e), (g2b, g2r_e), (b2b, b2r_e)):
            row = const.tile([1, D], fp32, tag="lnrow", name="lnrow")
            nc.sync.dma_start(out=row, in_=src_e[:, :])
            nc.gpsimd.partition_broadcast(dst, row[0:1, :])

        # attention-phase residents
        QtT = bigE.tile([P, ND, S], bf16)      # [o-part, o-blk, s]  ((Wq x + bq)/8)
        KtT = bigE.tile([P, ND, S], bf16)
        Vsb = bigE.tile([P, NQT, D], bf16)     # [k-part, k-blk, o]
        ctxT = resid.tile([P, ND, S], bf16)    # [(h,dh)-part, blk, q]
        xln = resid.tile([P, NQT, D], fp32)    # LN1 out [s-part, s-blk, d]

        # =========== Phase 1: projections ===========
        with (
            tc.tile_pool(name="p1x", bufs=1) as p1x,
            tc.tile_pool(name="p1w", bufs=3) as p1w,
            tc.tile_pool(name="p1ps", bufs=1, space="PSUM") as p1ps,
        ):
            qT = p1x.tile([P, ND, S], bf16)
            kvT = p1x.tile([P, ND, S], bf16)
            for dblk in range(ND):
                nc.sync.dma_start(
                    out=qT[:, dblk, :],
                    in_=qT_e[dblk * P : (dblk + 1) * P, :],
                )
                nc.sync.dma_start(
                    out=kvT[:, dblk, :],
                    in_=kvT_e[dblk * P : (dblk + 1) * P, :],
                )

            # Qt / Kt: psum[o-blk] [128, 512] += wT[d-blk][:, o-cols].T @ xT[d-blk]
            for wsrc, xsb, dst, bcol in (
                (wqT_e, qT, QtT, bqc),
                (wkT_e, kvT, KtT, bkc),
            ):
                psums = [p1ps.tile([P, S], fp32, tag=f"pp{i}", name=f"pp{i}") for i in range(ND)]
                for dblk in range(ND):
                    wch = p1w.tile([P, D], bf16, tag="wch")
                    nc.gpsimd.dma_start(
                        out=wch, in_=wsrc[dblk * P : (dblk + 1) * P, :]
                    )
                    for ob in range(ND):
                        nc.tensor.matmul(
                            psums[ob],
                            wch[:, ts(ob, P)],
                            xsb[:, dblk, :],
                            start=(dblk == 0),
                            stop=(dblk == ND - 1),
                        )
                for ob in range(ND):
                    nc.vector.tensor_scalar_add(
                        dst[:, ob, :], psums[ob], bcol[:, ob : ob + 1]
                    )

            # V: psum[(s-tile, o-half)] += kvT[d-blk][:, s-cols].T @ wvT[d-blk][:, o-half]
            vps = [
                [p1ps.tile([P, S], fp32, tag=f"pp{st * 2 + oh}", name=f"vp{st}{oh}") for oh in range(2)]
                for st in range(NQT)
            ]
            for st in range(NQT):
                for oh in range(2):
                    nc.tensor.matmul(
                        vps[st][oh], onesb, bvr[:, ts(oh, S)],
                        start=True, stop=False,
                    )
            for dblk in range(ND):
                wch = p1w.tile([P, D], bf16, tag="wch")
                nc.gpsimd.dma_start(out=wch, in_=wvT_e[dblk * P : (dblk + 1) * P, :])
                for st in range(NQT):
                    for oh in range(2):
                        nc.tensor.matmul(
                            vps[st][oh],
                            kvT[:, dblk, ts(st, P)],
                            wch[:, ts(oh, S)],
                            start=False,
                            stop=(dblk == ND - 1),
                        )
            for st in range(NQT):
                for oh in range(2):
                    nc.vector.tensor_copy(Vsb[:, st, ts(oh, S)], vps[st][oh])

        # =========== Phase 2+3: conv bias + attention, per qtile ===========
        with (
            tc.tile_pool(name="pbias", bufs=2) as pbias,
            tc.tile_pool(name="pcv", bufs=2) as pcv,
            tc.tile_pool(name="pcvps", bufs=1, space="PSUM") as pcvps,
            tc.tile_pool(name="pc2ps", bufs=1, space="PSUM") as pc2ps,
            tc.tile_pool(name="p3sb", bufs=2) as p3sb,
            tc.tile_pool(name="p3ps", bufs=1, space="PSUM") as p3ps,
        ):
            for blk in range(NQT // 2):
                # ---- conv bias for qtiles 2*blk, 2*blk+1 ----
                logms = {}
                for sub in range(2):
                    qt = blk * 2 + sub
                    for hemi in range(2):
                        qbase = qt * P + hemi * 64
                        amt = pcv.tile([P, NQI, S], fp32, tag="amt")
                        for g in range(8):
                            src = bass.AP(
                                tensor=amap_e,
                                offset=(1 + qbase + NQI * g) * AM + 1,
                                ap=[[AM * AM, CH], [AM, NQI], [1, S]],
                            )
                            nc.sync.dma_start(out=amt[CH * g : CH * (g + 1)], in_=src)
                        logm = pcv.tile([P, NQI * S], bf16, tag="logm", bufs=4)
                        nc.scalar.activation(
                            logm, amt.rearrange("p a b -> p (a b)"), AF.Ln,
                            bias=eps_log_c, scale=1.0,
                        )
                        logms[(sub, hemi)] = logm
                for sub in range(2):
                    qt = blk * 2 + sub
                    for hemi in range(2):
                        qbase = qt * P + hemi * 64
                        logm = logms[(sub, hemi)]
                        c2sb = pcv.tile([P, NQI, S], bf16, tag="c2sb", bufs=2)
                        for ch in range(NQI):
                            pAB = pcvps.tile([P, 2, S], fp32, tag="pAB", bufs=2)
                            for half, brow in ((0, bc1Ar), (1, bc1Br)):
                                nc.tensor.matmul(
                                    pAB[:, half, :], brow, ones5r,
                                    start=True, stop=False,
                                )
                            nc.tensor.matmul(
                                pAB[:, 0, :], c1A, logm[:, ts(ch, S)],
                                start=False, stop=True,
                            )
                            nc.tensor.matmul(
                                pAB[:, 1, :], c1B, logm[:, ts(ch, S)],
                                start=False, stop=True,
                            )
                            gAB = pcv.tile([P, 2, S], bf16, tag="gAB", bufs=2)
                            nc.scalar.activation(
                                gAB.rearrange("p a b -> p (a b)"),
                                pAB.rearrange("p a b -> p (a b)"),
                                AF.Gelu,
                            )
                            pC = pc2ps.tile([P, S], fp32, tag="pC")
                            nc.tensor.matmul(
                                pC, c2A, gAB[:, 0, :], start=True, stop=False
                            )
                            nc.tensor.matmul(
                                pC, c2B, gAB[:, 1, :], start=False, stop=True
                            )
                            nc.vector.tensor_scalar_add(c2sb[:, ch, :], pC, bc2c)
                        # scatter to DRAM bias stage in [q, h, k] order
                        for g in range(8):
                            q0 = qbase + NQI * g
                            nc.gpsimd.dma_start(
                                out=bstage[q0 : q0 + NQI].rearrange("i h k -> h i k"),
                                in_=c2sb[CH * g : CH * (g + 1)],
                            )

                for sub in range(2):
                    qt = blk * 2 + sub
                    biasq = pbias.tile([P, H, S], bf16, tag="biasq")
                    nc.sync.dma_start(
                        out=biasq.rearrange("p h k -> p (h k)"),
                        in_=bstage[qt * P : (qt + 1) * P].rearrange("q h k -> q (h k)"),
                    )
                    _attention_qt(nc, qt, biasq, QtT, KtT, Vsb, ctxT,
                                  ident_b, p3sb, p3ps)

        bigE_cm.__exit__(None, None, None)

        # =========== Phase 4: merge + residual + LN1 ===========
        with (
            tc.tile_pool(name="p4sb", bufs=2) as p4sb,
            tc.tile_pool(name="p4ps", bufs=1, space="PSUM") as p4ps,
        ):
            mps = [
                [p4ps.tile([P, S], fp32, tag=f"mp{st * 2 + oh}", name=f"mp{st}{oh}") for oh in range(2)]
                for st in range(NQT)
            ]
            for st in range(NQT):
                for oh in range(2):
                    nc.tensor.matmul(
                        mps[st][oh], onesb, bmr[:, ts(oh, S)], start=True, stop=False
                    )
            for dblk in range(ND):
                wch = p4sb.tile([P, D], bf16, tag="wch")
                nc.sync.dma_start(out=wch, in_=wmT_e[dblk * P : (dblk + 1) * P, :])
                for st in range(NQT):
                    for oh in range(2):
                        nc.tensor.matmul(
                            mps[st][oh],
                            ctxT[:, dblk, ts(st, P)],
                            wch[:, ts(oh, S)],
                            start=False,
                            stop=(dblk == ND - 1),
                        )
            for st in range(NQT):
                qtile = p4sb.tile([P, D], fp32, tag="qtile")
                nc.sync.dma_start(out=qtile, in_=qin_e[st * P : (st + 1) * P, :])
                x1 = p4sb.tile([P, D], fp32, tag="x1")
                for oh in range(2):
                    nc.vector.tensor_tensor(
                        out=x1[:, ts(oh, S)], in0=mps[st][oh],
                        in1=qtile[:, ts(oh, S)], op=ALU.add,
                    )
                _layernorm(nc, p4sb, xln[:, st, :], x1, g1b, b1b, eps_ln_c)

        bigL_cm = tc.tile_pool(name="bigL", bufs=1)
        bigL = bigL_cm.__enter__()
        fdt_ = fp8 if FFN_FP8 else bf16
        xlnT = bigL.tile([P, ND, S], fdt_)
        y1T = bigL.tile([P, NFF, S], fdt_)

        # =========== Phase 5: transpose x_ln ===========
        with tc.tile_pool(name="p5ps", bufs=2, space="PSUM") as p5ps:
            for dblk in range(ND):
                tp = p5ps.tile([P, S], fp32, tag="tp")
                for st in range(NQT):
                    nc.tensor.transpose(
                        tp[:, ts(st, P)], xln[:, st, ts(dblk, P)], ident_f
                    )
                nc.vector.tensor_copy(xlnT[:, dblk, :], tp)

        # =========== Phase 6: FFN1 + relu ===========
        with (
            tc.tile_pool(name="p6w", bufs=3) as p6w,
            tc.tile_pool(name="p6ps", bufs=2, space="PSUM") as p6ps,
        ):
            for ffb in range(NFF):
                fps = p6ps.tile([P, S], fp32, tag="fps")
                if FFN_FP8:
                    wf1c = p6w.tile([P, ND // 2, 2, P], fp8, tag="wf1c")
                    nc.sync.dma_start(
                        out=wf1c.rearrange("p a b c -> p (a b c)"), in_=wf1P_e[ffb]
                    )
                    for b in range(ND // 2):
                        nc.tensor.matmul(
                            fps,
                            wf1c[:, b, :, :],
                            xlnT[:, 2 * b : 2 * b + 2, :],
                            start=(b == 0), stop=(b == ND // 2 - 1),
                            perf_mode=DR,
                        )
                else:
                    wf1c = p6w.tile([P, ND, P], bf16, tag="wf1c")
                    nc.sync.dma_start(
                        out=wf1c.rearrange("p a b -> p (a b)"), in_=wf1P_e[ffb]
                    )
                    for dblk in range(ND):
                        nc.tensor.matmul(
                            fps,
                            wf1c[:, dblk, :],
                            xlnT[:, dblk, :],
                            start=(dblk == 0), stop=(dblk == ND - 1),
                        )
                nc.vector.tensor_scalar(
                    out=y1T[:, ffb, :], in0=fps,
                    scalar1=bf1c[:, ffb : ffb + 1], scalar2=0.0,
                    op0=ALU.add, op1=ALU.max,
                )

        # =========== Phase 7: FFN2 + residual + LN2 + out ===========
        with (
            tc.tile_pool(name="p7sb", bufs=2) as p7sb,
            tc.tile_pool(name="p7ps", bufs=1, space="PSUM") as p7ps,
        ):
            fps2 = [
                [p7ps.tile([P, S], fp32, tag=f"f2{st * 2 + oh}", name=f"f2{st}{oh}") for oh in range(2)]
                for st in range(NQT)
            ]
            for st in range(NQT):
                for oh in range(2):
                    nc.tensor.matmul(
                        fps2[st][oh], onesb, bf2r[:, ts(oh, S)],
                        start=True, stop=False,
                    )
            if FFN_FP8:
                for fb in range(NFF // 2):
                    wf2c = p7sb.tile([P, 2, D], fp8, tag="wch")
                    nc.sync.dma_start(
                        out=wf2c.rearrange("p a b -> p (a b)"), in_=wf2P_e[fb]
                    )
                    for st in range(NQT):
                        for oh in range(2):
                            nc.tensor.matmul(
                                fps2[st][oh],
                                y1T[:, 2 * fb : 2 * fb + 2, ts(st, P)],
                                wf2c[:, :, ts(oh, S)],
                                start=False,
                                stop=(fb == NFF // 2 - 1),
                                perf_mode=DR,
                            )
            else:
                for ffb in range(NFF):
                    wch = p7sb.tile([P, D], bf16, tag="wch")
                    nc.sync.dma_start(
                        out=wch,
                        in_=wf2P_e[ffb // 2, :, (ffb % 2) * D : (ffb % 2 + 1) * D],
                    )
                    for st in range(NQT):
                        for oh in range(2):
                            nc.tensor.matmul(
                                fps2[st][oh],
                                y1T[:, ffb, ts(st, P)],
                                wch[:, ts(oh, S)],
                                start=False,
                                stop=(ffb == NFF - 1),
                            )
            for st in range(NQT):
                x2 = p7sb.tile([P, D], fp32, tag="x2")
                for oh in range(2):
                    nc.vector.tensor_tensor(
                        out=x2[:, ts(oh, S)], in0=fps2[st][oh],
                        in1=xln[:, st, ts(oh, S)], op=ALU.add,
                    )
                xout = p7sb.tile([P, D], fp32, tag="xout")
                _layernorm(nc, p7sb, xout, x2, g2b, b2b, eps_ln_c)
                nc.sync.dma_start(out=out_e[st * P : (st + 1) * P, :], in_=xout)

        bigL_cm.__exit__(None, None, None)
        resid_cm.__exit__(None, None, None)
        dram_cm.__exit__(None, None, None)
        const_cm.__exit__(None, None, None)

    nc.finalize()
    return nc


def _prep_inputs(q, kv, attn_map, Wq, bq, Wk, bk, Wv, bv, Wm, bm,
                 Wc1, bc1, Wc2, bc2, Wf1, bf1, Wf2, bf2, g1, b1, g2, b2):
    """Host-side packing. Returns (shared dict, per-core list of dicts)."""
    f32 = np.float32
    bf = ml_dtypes.bfloat16

    def c(a):
        return np.ascontiguousarray(np.asarray(a), dtype=f32)

    def cb(a):
        return np.ascontiguousarray(np.asarray(a, dtype=f32)).astype(bf)

    Wq, Wk, Wv, Wm = c(Wq), c(Wk), c(Wv), c(Wm)
    Wc1, Wc2, Wf1, Wf2 = c(Wc1), c(Wc2), c(Wf1), c(Wf2)
    bq, bk, bv, bm = c(bq), c(bk), c(bv), c(bm)
    bc1, bc2, bf1, bf2 = c(bc1), c(bc2), c(bf1), c(bf2)
    g1, b1, g2, b2 = c(g1), c(b1), c(g2), c(b2)

    fdt = ml_dtypes.float8_e4m3 if FFN_FP8 else bf

    # FFN1 weights packed so each SBUF partition gets a contiguous row.
    # fp8/DoubleRow: wf1P[ffb, p, ((b,j),f)] = Wf1.T[(2b+j)*128+p, ffb*128+f]
    # bf16:          wf1P[ffb, p, (n,f)]     = Wf1.T[n*128+p, ffb*128+f]
    Wf1T = np.ascontiguousarray(Wf1.T)            # [D, FF]
    wf1P = np.ascontiguousarray(
        Wf1T.reshape(ND, P, NFF, P).transpose(2, 1, 0, 3).reshape(NFF, P, D)
    ).astype(fdt)
    # FFN2 weights: wf2P[fb, p, (j,d)] = Wf2.T[(2fb+j)*128+p, d]
    Wf2T = np.ascontiguousarray(Wf2.T)            # [FF, D]
    wf2P = np.ascontiguousarray(
        Wf2T.reshape(NFF // 2, 2, P, D).transpose(0, 2, 1, 3).reshape(NFF // 2, P, 2 * D)
    ).astype(fdt)

    shared = {
        "wqT": cb(Wq.T * 0.125), "wkT": cb(Wk.T), "wvT": cb(Wv.T),
        "wmT": cb(Wm.T),
        "wf1P": wf1P,
        "wf2P": wf2P,
        "bqc": c((bq / 8.0).reshape(ND, P).T),
        "bkc": c(bk.reshape(ND, P).T),
        "bf1c": c(bf1.reshape(NFF, P).T),
        "bvr": cb(bv.reshape(1, D)), "bmr": cb(bm.reshape(1, D)),
        "bf2r": cb(bf2.reshape(1, D)),
        "onesb": np.ones((1, P), bf),
        "g1r": g1.reshape(1, D), "b1r": b1.reshape(1, D),
        "g2r": g2.reshape(1, D), "b2r": b2.reshape(1, D),
    }
    # conv block-diag lhsT [K, M]: out[(g,oh)] = sum_c lhsT[(g,c),(g,oh)] rhs[(g,c)]
    c1A = np.zeros((P, P), f32)
    c1B = np.zeros((P, P), f32)
    c2A = np.zeros((P, P), f32)
    c2B = np.zeros((P, P), f32)
    for g in range(8):
        sl = slice(g * 16, g * 16 + 16)
        c1A[sl, sl] = Wc1[0:16, :].T     # [c, oh]
        c1B[sl, sl] = Wc1[16:32, :].T
        c2A[sl, sl] = Wc2[:, 0:16].T     # [ci, h]
        c2B[sl, sl] = Wc2[:, 16:32].T
    shared["c1A"] = c1A.astype(bf)
    shared["c1B"] = c1B.astype(bf)
    shared["c2A"] = c2A.astype(bf)
    shared["c2B"] = c2B.astype(bf)
    shared["bc1Ar"] = np.tile(bc1[0:16], 8).reshape(1, P).astype(bf)
    shared["bc1Br"] = np.tile(bc1[16:32], 8).reshape(1, P).astype(bf)
    shared["ones5r"] = np.ones((1, S), bf)
    shared["bc2c"] = np.tile(bc2, 8).reshape(P, 1).astype(f32)

    q = c(q)
    kv = c(kv)
    attn_map = np.asarray(attn_map)
    per_core = []
    for b in range(B):
        per_core.append({
            "qT": np.ascontiguousarray(q[b].T).astype(bf),
            "kvT": np.ascontiguousarray(kv[b].T).astype(bf),
            "qin": q[b],
            "amap": c(attn_map[b]),
        })
    return shared, per_core


def kernel(**inputs):
    if "nc" not in _CACHED:
        _CACHED["nc"] = build_program()
    nc = _CACHED["nc"]
    shared, per_core = _prep_inputs(**inputs)
    in_maps = [dict(shared, **pc) for pc in per_core]
    res = run_bass_kernel_spmd(nc, in_maps, list(range(B)))
    out = np.stack([res.results[i]["out"] for i in range(B)], axis=0)
    return out.astype(np.float32)


# revision 25
# speedup vs baseline: 1.0785x; 1.0785x over previous
"""Trainium2 Bass kernel for nn_CrossAttention (dense transformer block).

Sharding: data-parallel over batch — 8 batch elements, one per NeuronCore.
Each core runs the full block for its batch element:
  bias = Conv1x1(gelu(Conv1x1(log(attn_map[1:,1:] + eps))))
  MHA(q, kv) with bias added to scores; residual + LN; FFN; residual + LN.

v2: bf16 weights/activations on the matmul path, conv-bias pipeline with
batched scalar ops (log+exp share one ACT table set), head-pair packed
attention with DMA-xbar transposes, epilogues on vector/gpsimd.

Self-contained: hardcodes all shapes; host-side numpy prepares transposed /
packed weight layouts per core.
"""

import numpy as np
import ml_dtypes

import concourse.bass as bass
import concourse.mybir as mybir
import concourse.tile as tile
from concourse.tile import add_dep_helper
from concourse import bacc
from concourse.bass import ts
from concourse.bass_utils import run_bass_kernel_spmd
from concourse.masks import make_identity

AF = mybir.ActivationFunctionType
ALU = mybir.AluOpType
AX = mybir.AxisListType

B, S, D, H, DH, FF = 8, 512, 1024, 16, 64, 4096
CH, CHID = 16, 32
EPS_LOG = 1e-6
EPS_LN = 1e-6
P = 128
NQT = S // P          # 4 q-tiles
ND = D // P           # 8 d-blocks
NFF = FF // P         # 32 ff-blocks
AM = 513              # attn_map edge
NQI = 8               # q rows per conv group (8 groups of 16 partitions/hemi)

fp32 = mybir.dt.float32
bf16 = mybir.dt.bfloat16
fp8 = mybir.dt.float8e4
DR = mybir.MatmulPerfMode.DoubleRow

# fp8 e4m3 for both FFN matmuls (DoubleRow, 2 MACs/cell/cycle).
# Measured: pushes rel err to 2.8e-2 (> 2e-2 gate) — e4m3's 3 mantissa
# bits give ~2-3% rms weight error. Keep bf16.
FFN_FP8 = False

# 'dma' = xbar DMA transpose for attention probs, 'pe' = tensor-engine
ATT_TRANSPOSE = "dma"

_CACHED = {}


def _layernorm(nc, pool, out_ap, x_ap, gb, bb, eps_c):
    """out = (x - mean(x)) * rsqrt(var(x) + eps) * g + b over free dim (D)."""
    nsub = D // 512
    stats = pool.tile([P, nsub, nc.vector.BN_STATS_DIM], fp32, tag="ln_stats")
    for i in range(nsub):
        nc.vector.bn_stats(out=stats[:, i, :], in_=x_ap[:, ts(i, 512)])
    mv = pool.tile([P, nc.vector.BN_AGGR_DIM], fp32, tag="ln_mv")
    nc.vector.bn_aggr(out=mv, in_=stats)
    rstd = pool.tile([P, 1], fp32, tag="ln_rstd")
    nc.scalar.activation(rstd, mv[:, 1:2], AF.Sqrt, bias=eps_c, scale=1.0)
    nc.vector.reciprocal(out=rstd, in_=rstd)
    u = pool.tile([P, D], fp32, tag="ln_u")
    nc.vector.scalar_tensor_tensor(
        out=u, in0=x_ap, scalar=mv[:, 0:1], in1=gb,
        op0=ALU.subtract, op1=ALU.mult,
    )
    nc.vector.scalar_tensor_tensor(
        out=out_ap, in0=u, scalar=rstd[:, 0:1], in1=bb,
        op0=ALU.mult, op1=ALU.add,
    )


def _attention_qt(nc, qt, biasq, QtT, KtT, Vsb, ctxT, ident_b, p3sb, p3ps,
                  exp_insts):
    """Attention for one 128-row q-tile, bias already staged in biasq."""
    den = p3sb.tile([P, H], fp32, tag="den")
    rec = p3sb.tile([P, H], fp32, tag="rec")
    for hp in range(8):  # 8 head pairs
        sc2 = p3ps.tile([P, 2, S], fp32, tag="sc2")
        for j in range(2):
            h = hp * 2 + j
            hb, ho = (h * DH) // P, (h * DH) % P
            nc.tensor.matmul(
                sc2[:, j, :],
                QtT[ho : ho + DH, hb, ts(qt, P)],
                KtT[ho : ho + DH, hb, :],
                start=True, stop=False,
            )
        for j in range(2):
            h = hp * 2 + j
            nc.tensor.matmul(
                sc2[:, j, :], ident_b, biasq[:, h, :],
                start=False, stop=True,
            )
        att2 = p3sb.tile([P, 2, S], bf16, tag="att2")
        exp_insts.append(nc.scalar.activation(
            att2.rearrange("p a b -> p (a b)"),
            sc2.rearrange("p a b -> p (a b)"),
            AF.Exp,
        ))
        nc.vector.tensor_reduce(
            out=den[:, hp * 2 : hp * 2 + 2], in_=att2,
            axis=AX.X, op=ALU.add,
        )
        nc.vector.reciprocal(
            out=rec[:, hp * 2 : hp * 2 + 2],
            in_=den[:, hp * 2 : hp * 2 + 2],
        )
        cx = p3ps.tile([P, P], fp32, tag="cx")
        for j in range(2):
            h = hp * 2 + j
            attn = p3sb.tile([P, S], bf16, tag="attn", bufs=3)
            nc.vector.tensor_scalar_mul(
                attn, att2[:, j, :], rec[:, h : h + 1]
            )
            atTs = p3sb.tile([P, NQT, P], bf16, tag="atTs", bufs=3)
            if ATT_TRANSPOSE == "dma":
                nc.sync.dma_start_transpose(atTs, attn)
            else:
                atT_ps = p3ps.tile([P, S], bf16, tag="atT")
                for kt in range(NQT):
                    nc.tensor.transpose(
                        atT_ps[:, ts(kt, P)], attn[:, ts(kt, P)], ident_b
                    )
                nc.vector.tensor_copy(
                    atTs.rearrange("p a b -> p (a b)"), atT_ps
                )
            for kt in range(NQT):
                nc.tensor.matmul(
                    cx[j * DH : (j + 1) * DH, :],
                    Vsb[:, kt, h * DH : (h + 1) * DH],
                    atTs[:, kt, :],
                    start=(kt == 0), stop=(kt == NQT - 1),
                    tile_position=(0, j * DH),
                )
        nc.vector.tensor_copy(ctxT[:, hp, ts(qt, P)], cx)


def build_program():
    nc = bacc.Bacc(None)

    # ---------------- DRAM I/O ----------------
    qT_e = nc.dram_tensor("qT", [D, S], bf16, kind="ExternalInput")
    kvT_e = nc.dram_tensor("kvT", [D, S], bf16, kind="ExternalInput")
    qin_e = nc.dram_tensor("qin", [S, D], fp32, kind="ExternalInput")
    amap_e = nc.dram_tensor("amap", [CH, AM, AM], fp32, kind="ExternalInput")
    wqT_e = nc.dram_tensor("wqT", [D, D], bf16, kind="ExternalInput")
    wkT_e = nc.dram_tensor("wkT", [D, D], bf16, kind="ExternalInput")
    wvT_e = nc.dram_tensor("wvT", [D, D], bf16, kind="ExternalInput")
    wmT_e = nc.dram_tensor("wmT", [D, D], bf16, kind="ExternalInput")
    fdt = fp8 if FFN_FP8 else bf16
    # FFN1 weights: [ffb, p, (dpair b, j, f)] = Wf1.T[(2b+j)*128+p, ffb*128+f]
    wf1P_e = nc.dram_tensor("wf1P", [NFF, P, D], fdt, kind="ExternalInput")
    # FFN2 weights: [fb, p, (j, d)] = Wf2.T[(2fb+j)*128+p, d]
    wf2P_e = nc.dram_tensor("wf2P", [NFF // 2, P, 2 * D], fdt, kind="ExternalInput")
    c1A_e = nc.dram_tensor("c1A", [P, P], bf16, kind="ExternalInput")
    c1B_e = nc.dram_tensor("c1B", [P, P], bf16, kind="ExternalInput")
    c2A_e = nc.dram_tensor("c2A", [P, P], bf16, kind="ExternalInput")
    c2B_e = nc.dram_tensor("c2B", [P, P], bf16, kind="ExternalInput")
    # per-partition bias columns
    bqc_e = nc.dram_tensor("bqc", [P, ND], fp32, kind="ExternalInput")   # bq/8
    bkc_e = nc.dram_tensor("bkc", [P, ND], fp32, kind="ExternalInput")
    bc1A_e = nc.dram_tensor("bc1A", [P, 1], fp32, kind="ExternalInput")
    bc1B_e = nc.dram_tensor("bc1B", [P, 1], fp32, kind="ExternalInput")
    bc2c_e = nc.dram_tensor("bc2c", [P, 1], fp32, kind="ExternalInput")
    bf1c_e = nc.dram_tensor("bf1c", [P, NFF], fp32, kind="ExternalInput")
    # bias rows (K=1 matmul trick)
    bvr_e = nc.dram_tensor("bvr", [1, D], bf16, kind="ExternalInput")
    bmr_e = nc.dram_tensor("bmr", [1, D], bf16, kind="ExternalInput")
    bf2r_e = nc.dram_tensor("bf2r", [1, D], bf16, kind="ExternalInput")
    onesb_e = nc.dram_tensor("onesb", [1, P], bf16, kind="ExternalInput")
    # LN params as rows
    g1r_e = nc.dram_tensor("g1r", [1, D], fp32, kind="ExternalInput")
    b1r_e = nc.dram_tensor("b1r", [1, D], fp32, kind="ExternalInput")
    g2r_e = nc.dram_tensor("g2r", [1, D], fp32, kind="ExternalInput")
    b2r_e = nc.dram_tensor("b2r", [1, D], fp32, kind="ExternalInput")

    out_e = nc.dram_tensor("out", [S, D], fp32, kind="ExternalOutput")

    with tile.TileContext(nc) as tc:
        # ------------- persistent pools -------------
        const_cm = tc.tile_pool(name="const", bufs=1)
        const = const_cm.__enter__()
        dram_cm = tc.tile_pool(name="dstage", bufs=1, space="DRAM")
        dram = dram_cm.__enter__()
        bstage = dram.tile([S, H, S], bf16)
        resid_cm = tc.tile_pool(name="resid", bufs=1)  # ctxT/xln (ph3-7)
        resid = resid_cm.__enter__()
        bigE_cm = tc.tile_pool(name="bigE", bufs=1)   # Qt/Kt/V (ph1-3)
        bigE = bigE_cm.__enter__()

        ident_b = const.tile([P, P], bf16)
        make_identity(nc, ident_b)
        ident_f = const.tile([P, P], fp32)
        make_identity(nc, ident_f)

        eps_log_c = const.tile([P, 1], fp32)
        nc.vector.memset(eps_log_c, EPS_LOG)
        eps_ln_c = const.tile([P, 1], fp32)
        nc.vector.memset(eps_ln_c, EPS_LN)

        c1A = const.tile([P, P], bf16)
        c1B = const.tile([P, P], bf16)
        c2A = const.tile([P, P], bf16)
        c2B = const.tile([P, P], bf16)
        nc.sync.dma_start(out=c1A, in_=c1A_e[:, :])
        nc.sync.dma_start(out=c1B, in_=c1B_e[:, :])
        nc.sync.dma_start(out=c2A, in_=c2A_e[:, :])
        nc.sync.dma_start(out=c2B, in_=c2B_e[:, :])
        bc1A = const.tile([P, 1], fp32)
        bc1B = const.tile([P, 1], fp32)
        bc2c = const.tile([P, 1], fp32)
        nc.sync.dma_start(out=bc1A, in_=bc1A_e[:, :])
        nc.sync.dma_start(out=bc1B, in_=bc1B_e[:, :])
        nc.sync.dma_start(out=bc2c, in_=bc2c_e[:, :])
        bqc = const.tile([P, ND], fp32)
        bkc = const.tile([P, ND], fp32)
        bf1c = const.tile([P, NFF], fp32)
        nc.sync.dma_start(out=bqc, in_=bqc_e[:, :])
        nc.sync.dma_start(out=bkc, in_=bkc_e[:, :])
        nc.sync.dma_start(out=bf1c, in_=bf1c_e[:, :])
        bvr = const.tile([1, D], bf16)
        bmr = const.tile([1, D], bf16)
        bf2r = const.tile([1, D], bf16)
        onesb = const.tile([1, P], bf16)
        nc.sync.dma_start(out=bvr, in_=bvr_e[:, :])
        nc.sync.dma_start(out=bmr, in_=bmr_e[:, :])
        nc.sync.dma_start(out=bf2r, in_=bf2r_e[:, :])
        nc.sync.dma_start(out=onesb, in_=onesb_e[:, :])

        # LN param broadcast tiles [128, D]
        g1b = const.tile([P, D], fp32)
        b1b = const.tile([P, D], fp32)
        g2b = const.tile([P, D], fp32)
        b2b = const.tile([P, D], fp32)
        for dst, src_e in ((g1b, g1r_e), (b1b, b1r_e), (g2b, g2r_e), (b2b, b2r_e)):
            row = const.tile([1, D], fp32, tag="lnrow", name="lnrow")
            nc.sync.dma_start(out=row, in_=src_e[:, :])
            nc.gpsimd.partition_broadcast(dst, row[0:1, :])

        # attention-phase residents
        QtT = bigE.tile([P, ND, S], bf16)      # [o-part, o-blk, s]  ((Wq x + bq)/8)
        KtT = bigE.tile([P, ND, S], bf16)
        Vsb = bigE.tile([P, NQT, D], bf16)     # [k-part, k-blk, o]
        ctxT = resid.tile([P, ND, S], bf16)    # [(h,dh)-part, blk, q]
        xln = resid.tile([P, NQT, D], fp32)    # LN1 out [s-part, s-blk, d]

        # =========== Phase 1: projections ===========
        with (
            tc.tile_pool(name="p1x", bufs=1) as p1x,
            tc.tile_pool(name="p1w", bufs=3) as p1w,
            tc.tile_pool(name="p1ps", bufs=1, space="PSUM") as p1ps,
        ):
            qT = p1x.tile([P, ND, S], bf16)
            kvT = p1x.tile([P, ND, S], bf16)
            for dblk in range(ND):
                nc.sync.dma_start(
                    out=qT[:, dblk, :],
                    in_=qT_e[dblk * P : (dblk + 1) * P, :],
                )
                nc.sync.dma_start(
                    out=kvT[:, dblk, :],
                    in_=kvT_e[dblk * P : (dblk + 1) * P, :],
                )

            # Qt / Kt: psum[o-blk] [128, 512] += wT[d-blk][:, o-cols].T @ xT[d-blk]
            for wsrc, xsb, dst, bcol in (
                (wqT_e, qT, QtT, bqc),
                (wkT_e, kvT, KtT, bkc),
            ):
                psums = [p1ps.tile([P, S], fp32, tag=f"pp{i}", name=f"pp{i}") for i in range(ND)]
                for dblk in range(ND):
                    wch = p1w.tile([P, D], bf16, tag="wch")
                    nc.sync.dma_start(
                        out=wch, in_=wsrc[dblk * P : (dblk + 1) * P, :]
                    )
                    for ob in range(ND):
                        nc.tensor.matmul(
                            psums[ob],
                            wch[:, ts(ob, P)],
                            xsb[:, dblk, :],
                            start=(dblk == 0),
                            stop=(dblk == ND - 1),
                        )
                for ob in range(ND):
                    nc.vector.tensor_scalar_add(
                        dst[:, ob, :], psums[ob], bcol[:, ob : ob + 1]
                    )

            # V: psum[(s-tile, o-half)] += kvT[d-blk][:, s-cols].T @ wvT[d-blk][:, o-half]
            vps = [
                [p1ps.tile([P, S], fp32, tag=f"pp{st * 2 + oh}", name=f"vp{st}{oh}") for oh in range(2)]
                for st in range(NQT)
            ]
            for st in range(NQT):
                for oh in range(2):
                    nc.tensor.matmul(
                        vps[st][oh], onesb, bvr[:, ts(oh, S)],
                        start=True, stop=False,
                    )
            for dblk in range(ND):
                wch = p1w.tile([P, D], bf16, tag="wch")
                nc.sync.dma_start(out=wch, in_=wvT_e[dblk * P : (dblk + 1) * P, :])
                for st in range(NQT):
                    for oh in range(2):
                        nc.tensor.matmul(
                            vps[st][oh],
                            kvT[:, dblk, ts(st, P)],
                            wch[:, ts(oh, S)],
                            start=False,
                            stop=(dblk == ND - 1),
                        )
            for st in range(NQT):
                for oh in range(2):
                    nc.vector.tensor_copy(Vsb[:, st, ts(oh, S)], vps[st][oh])

        # =========== Phase 2+3: conv bias + attention, per qtile ===========
        with (
            tc.tile_pool(name="pbias", bufs=2) as pbias,
            tc.tile_pool(name="pcv", bufs=2) as pcv,
            tc.tile_pool(name="pcvps", bufs=1, space="PSUM") as pcvps,
            tc.tile_pool(name="pc2ps", bufs=1, space="PSUM") as pc2ps,
            tc.tile_pool(name="p3sb", bufs=2) as p3sb,
            tc.tile_pool(name="p3ps", bufs=1, space="PSUM") as p3ps,
        ):
            prev_last_exp = None
            for blk in range(NQT // 2):
                # ---- conv bias for qtiles 2*blk, 2*blk+1 ----
                logms = {}
                first_log = None
                last_gelu = None
                for sub in range(2):
                    qt = blk * 2 + sub
                    for hemi in range(2):
                        qbase = qt * P + hemi * 64
                        amt = pcv.tile([P, NQI, S], fp32, tag="amt")
                        for g in range(8):
                            src = bass.AP(
                                tensor=amap_e,
                                offset=(1 + qbase + NQI * g) * AM + 1,
                                ap=[[AM * AM, CH], [AM, NQI], [1, S]],
                            )
                            nc.sync.dma_start(out=amt[CH * g : CH * (g + 1)], in_=src)
                        logm = pcv.tile([P, NQI * S], bf16, tag="logm", bufs=4)
                        li = nc.scalar.activation(
                            logm, amt.rearrange("p a b -> p (a b)"), AF.Ln,
                            bias=eps_log_c, scale=1.0,
                        )
                        if first_log is None:
                            first_log = li
                        logms[(sub, hemi)] = logm
                # keep the ACT table from ping-ponging between the Ln/Exp set
                # and the Gelu set: this block's logs come after the previous
                # block's last softmax exp
                if prev_last_exp is not None and first_log is not None:
                    add_dep_helper(first_log.ins, prev_last_exp.ins, False,
                                   "act-table grouping")
                for sub in range(2):
                    qt = blk * 2 + sub
                    for hemi in range(2):
                        qbase = qt * P + hemi * 64
                        logm = logms[(sub, hemi)]
                        c2sb = pcv.tile([P, NQI, S], bf16, tag="c2sb", bufs=2)
                        for ch in range(NQI):
                            pAB = pcvps.tile([P, 2, S], fp32, tag="pAB", bufs=2)
                            nc.tensor.matmul(
                                pAB[:, 0, :], c1A, logm[:, ts(ch, S)],
                                start=True, stop=True,
                            )
                            nc.tensor.matmul(
                                pAB[:, 1, :], c1B, logm[:, ts(ch, S)],
                                start=True, stop=True,
                            )
                            gAB = pcv.tile([P, 2, S], bf16, tag="gAB", bufs=2)
                            nc.scalar.activation(
                                gAB[:, 0, :], pAB[:, 0, :], AF.Gelu,
                                bias=bc1A, scale=1.0,
                            )
                            last_gelu = nc.scalar.activation(
                                gAB[:, 1, :], pAB[:, 1, :], AF.Gelu,
                                bias=bc1B, scale=1.0,
                            )
                            pC = pc2ps.tile([P, S], fp32, tag="pC")
                            nc.tensor.matmul(
                                pC, c2A, gAB[:, 0, :], start=True, stop=False
                            )
                            nc.tensor.matmul(
                                pC, c2B, gAB[:, 1, :], start=False, stop=True
                            )
                            nc.vector.tensor_scalar_add(c2sb[:, ch, :], pC, bc2c)
                        # scatter to DRAM bias stage in [q, h, k] order
                        for g in range(8):
                            q0 = qbase + NQI * g
                            nc.gpsimd.dma_start(
                                out=bstage[q0 : q0 + NQI].rearrange("i h k -> h i k"),
                                in_=c2sb[CH * g : CH * (g + 1)],
                            )

                exp_insts = []
                for sub in range(2):
                    qt = blk * 2 + sub
                    biasq = pbias.tile([P, H, S], bf16, tag="biasq")
                    nc.sync.dma_start(
                        out=biasq.rearrange("p h k -> p (h k)"),
                        in_=bstage[qt * P : (qt + 1) * P].rearrange("q h k -> q (h k)"),
                    )
                    _attention_qt(nc, qt, biasq, QtT, KtT, Vsb, ctxT,
                                  ident_b, p3sb, p3ps, exp_insts)
                # this block's exps come after this block's last gelu
                if last_gelu is not None and exp_insts:
                    add_dep_helper(exp_insts[0].ins, last_gelu.ins, False,
                                   "act-table grouping")
                prev_last_exp = exp_insts[-1] if exp_insts else None

        bigE_cm.__exit__(None, None, None)

        # =========== Phase 4: merge + residual + LN1 ===========
        with (
            tc.tile_pool(name="p4sb", bufs=2) as p4sb,
            tc.tile_pool(name="p4ps", bufs=1, space="PSUM") as p4ps,
        ):
            mps = [
                [p4ps.tile([P, S], fp32, tag=f"mp{st * 2 + oh}", name=f"mp{st}{oh}") for oh in range(2)]
                for st in range(NQT)
            ]
            for st in range(NQT):
                for oh in range(2):
                    nc.tensor.matmul(
                        mps[st][oh], onesb, bmr[:, ts(oh, S)], start=True, stop=False
                    )
            for dblk in range(ND):
                wch = p4sb.tile([P, D], bf16, tag="wch")
                nc.sync.dma_start(out=wch, in_=wmT_e[dblk * P : (dblk + 1) * P, :])
                for st in range(NQT):
                    for oh in range(2):
                        nc.tensor.matmul(
                            mps[st][oh],
                            ctxT[:, dblk, ts(st, P)],
                            wch[:, ts(oh, S)],
                            start=False,
                            stop=(dblk == ND - 1),
                        )
            for st in range(NQT):
                qtile = p4sb.tile([P, D], fp32, tag="qtile")
                nc.sync.dma_start(out=qtile, in_=qin_e[st * P : (st + 1) * P, :])
                x1 = p4sb.tile([P, D], fp32, tag="x1")
                for oh in range(2):
                    nc.vector.tensor_tensor(
                        out=x1[:, ts(oh, S)], in0=mps[st][oh],
                        in1=qtile[:, ts(oh, S)], op=ALU.add,
                    )
                _layernorm(nc, p4sb, xln[:, st, :], x1, g1b, b1b, eps_ln_c)

        bigL_cm = tc.tile_pool(name="bigL", bufs=1)
        bigL = bigL_cm.__enter__()
        fdt_ = fp8 if FFN_FP8 else bf16
        xlnT = bigL.tile([P, ND, S], fdt_)
        y1T = bigL.tile([P, NFF, S], fdt_)

        # =========== Phase 5: transpose x_ln ===========
        with tc.tile_pool(name="p5ps", bufs=2, space="PSUM") as p5ps:
            for dblk in range(ND):
                tp = p5ps.tile([P, S], fp32, tag="tp")
                for st in range(NQT):
                    nc.tensor.transpose(
                        tp[:, ts(st, P)], xln[:, st, ts(dblk, P)], ident_f
                    )
                nc.vector.tensor_copy(xlnT[:, dblk, :], tp)

        # =========== Phase 6: FFN1 + relu ===========
        with (
            tc.tile_pool(name="p6w", bufs=3) as p6w,
            tc.tile_pool(name="p6ps", bufs=2, space="PSUM") as p6ps,
        ):
            for ffb in range(NFF):
                fps = p6ps.tile([P, S], fp32, tag="fps")
                if FFN_FP8:
                    wf1c = p6w.tile([P, ND // 2, 2, P], fp8, tag="wf1c")
                    nc.sync.dma_start(
                        out=wf1c.rearrange("p a b c -> p (a b c)"), in_=wf1P_e[ffb]
                    )
                    for b in range(ND // 2):
                        nc.tensor.matmul(
                            fps,
                            wf1c[:, b, :, :],
                            xlnT[:, 2 * b : 2 * b + 2, :],
                            start=(b == 0), stop=(b == ND // 2 - 1),
                            perf_mode=DR,
                        )
                else:
                    wf1c = p6w.tile([P, ND, P], bf16, tag="wf1c")
                    nc.sync.dma_start(
                        out=wf1c.rearrange("p a b -> p (a b)"), in_=wf1P_e[ffb]
                    )
                    for dblk in range(ND):
                        nc.tensor.matmul(
                            fps,
                            wf1c[:, dblk, :],
                            xlnT[:, dblk, :],
                            start=(dblk == 0), stop=(dblk == ND - 1),
                        )
                nc.vector.tensor_scalar(
                    out=y1T[:, ffb, :], in0=fps,
                    scalar1=bf1c[:, ffb : ffb + 1], scalar2=0.0,
                    op0=ALU.add, op1=ALU.max,
                )

        # =========== Phase 7: FFN2 + residual + LN2 + out ===========
        with (
            tc.tile_pool(name="p7sb", bufs=2) as p7sb,
            tc.tile_pool(name="p7ps", bufs=1, space="PSUM") as p7ps,
        ):
            fps2 = [
                [p7ps.tile([P, S], fp32, tag=f"f2{st * 2 + oh}", name=f"f2{st}{oh}") for oh in range(2)]
                for st in range(NQT)
            ]
            for st in range(NQT):
                for oh in range(2):
                    nc.tensor.matmul(
                        fps2[st][oh], onesb, bf2r[:, ts(oh, S)],
                        start=True, stop=False,
                    )
            if FFN_FP8:
                for fb in range(NFF // 2):
                    wf2c = p7sb.tile([P, 2, D], fp8, tag="wch")
                    nc.sync.dma_start(
                        out=wf2c.rearrange("p a b -> p (a b)"), in_=wf2P_e[fb]
                    )
                    for st in range(NQT):
                        for oh in range(2):
                            nc.tensor.matmul(
                                fps2[st][oh],
                                y1T[:, 2 * fb : 2 * fb + 2, ts(st, P)],
                                wf2c[:, :, ts(oh, S)],
                                start=False,
                                stop=(fb == NFF // 2 - 1),
                                perf_mode=DR,
                            )
            else:
                for ffb in range(NFF):
                    wch = p7sb.tile([P, D], bf16, tag="wch")
                    nc.sync.dma_start(
                        out=wch,
                        in_=wf2P_e[ffb // 2, :, (ffb % 2) * D : (ffb % 2 + 1) * D],
                    )
                    for st in range(NQT):
                        for oh in range(2):
                            nc.tensor.matmul(
                                fps2[st][oh],
                                y1T[:, ffb, ts(st, P)],
                                wch[:, ts(oh, S)],
                                start=False,
                                stop=(ffb == NFF - 1),
                            )
            for st in range(NQT):
                x2 = p7sb.tile([P, D], fp32, tag="x2")
                for oh in range(2):
                    nc.vector.tensor_tensor(
                        out=x2[:, ts(oh, S)], in0=fps2[st][oh],
                        in1=xln[:, st, ts(oh, S)], op=ALU.add,
                    )
                xout = p7sb.tile([P, D], fp32, tag="xout")
                _layernorm(nc, p7sb, xout, x2, g2b, b2b, eps_ln_c)
                nc.sync.dma_start(out=out_e[st * P : (st + 1) * P, :], in_=xout)

        bigL_cm.__exit__(None, None, None)
        resid_cm.__exit__(None, None, None)
        dram_cm.__exit__(None, None, None)
        const_cm.__exit__(None, None, None)

    nc.finalize()
    return nc


def _prep_inputs(q, kv, attn_map, Wq, bq, Wk, bk, Wv, bv, Wm, bm,
                 Wc1, bc1, Wc2, bc2, Wf1, bf1, Wf2, bf2, g1, b1, g2, b2):
    """Host-side packing. Returns (shared dict, per-core list of dicts)."""
    f32 = np.float32
    bf = ml_dtypes.bfloat16

    def c(a):
        return np.ascontiguousarray(np.asarray(a), dtype=f32)

    def cb(a):
        return np.ascontiguousarray(np.asarray(a, dtype=f32)).astype(bf)

    Wq, Wk, Wv, Wm = c(Wq), c(Wk), c(Wv), c(Wm)
    Wc1, Wc2, Wf1, Wf2 = c(Wc1), c(Wc2), c(Wf1), c(Wf2)
    bq, bk, bv, bm = c(bq), c(bk), c(bv), c(bm)
    bc1, bc2, bf1, bf2 = c(bc1), c(bc2), c(bf1), c(bf2)
    g1, b1, g2, b2 = c(g1), c(b1), c(g2), c(b2)

    fdt = ml_dtypes.float8_e4m3 if FFN_FP8 else bf

    # FFN1 weights packed so each SBUF partition gets a contiguous row.
    # fp8/DoubleRow: wf1P[ffb, p, ((b,j),f)] = Wf1.T[(2b+j)*128+p, ffb*128+f]
    # bf16:          wf1P[ffb, p, (n,f)]     = Wf1.T[n*128+p, ffb*128+f]
    Wf1T = np.ascontiguousarray(Wf1.T)            # [D, FF]
    wf1P = np.ascontiguousarray(
        Wf1T.reshape(ND, P, NFF, P).transpose(2, 1, 0, 3).reshape(NFF, P, D)
    ).astype(fdt)
    # FFN2 weights: wf2P[fb, p, (j,d)] = Wf2.T[(2fb+j)*128+p, d]
    Wf2T = np.ascontiguousarray(Wf2.T)            # [FF, D]
    wf2P = np.ascontiguousarray(
        Wf2T.reshape(NFF // 2, 2, P, D).transpose(0, 2, 1, 3).reshape(NFF // 2, P, 2 * D)
    ).astype(fdt)

    shared = {
        "wqT": cb(Wq.T * 0.125), "wkT": cb(Wk.T), "wvT": cb(Wv.T),
        "wmT": cb(Wm.T),
        "wf1P": wf1P,
        "wf2P": wf2P,
        "bqc": c((bq / 8.0).reshape(ND, P).T),
        "bkc": c(bk.reshape(ND, P).T),
        "bf1c": c(bf1.reshape(NFF, P).T),
        "bvr": cb(bv.reshape(1, D)), "bmr": cb(bm.reshape(1, D)),
        "bf2r": cb(bf2.reshape(1, D)),
        "onesb": np.ones((1, P), bf),
        "g1r": g1.reshape(1, D), "b1r": b1.reshape(1, D),
        "g2r": g2.reshape(1, D), "b2r": b2.reshape(1, D),
    }
    # conv block-diag lhsT [K, M]: out[(g,oh)] = sum_c lhsT[(g,c),(g,oh)] rhs[(g,c)]
    c1A = np.zeros((P, P), f32)
    c1B = np.zeros((P, P), f32)
    c2A = np.zeros((P, P), f32)
    c2B = np.zeros((P, P), f32)
    for g in range(8):
        sl = slice(g * 16, g * 16 + 16)
        c1A[sl, sl] = Wc1[0:16, :].T     # [c, oh]
        c1B[sl, sl] = Wc1[16:32, :].T
        c2A[sl, sl] = Wc2[:, 0:16].T     # [ci, h]
        c2B[sl, sl] = Wc2[:, 16:32].T
    shared["c1A"] = c1A.astype(bf)
    shared["c1B"] = c1B.astype(bf)
    shared["c2A"] = c2A.astype(bf)
    shared["c2B"] = c2B.astype(bf)
    shared["bc1A"] = np.tile(bc1[0:16], 8).reshape(P, 1).astype(f32)
    shared["bc1B"] = np.tile(bc1[16:32], 8).reshape(P, 1).astype(f32)
    shared["bc2c"] = np.tile(bc2, 8).reshape(P, 1).astype(f32)

    q = c(q)
    kv = c(kv)
    attn_map = np.asarray(attn_map)
    per_core = []
    for b in range(B):
        per_core.append({
            "qT": np.ascontiguousarray(q[b].T).astype(bf),
            "kvT": np.ascontiguousarray(kv[b].T).astype(bf),
            "qin": q[b],
            "amap": c(attn_map[b]),
        })
    return shared, per_core


def kernel(**inputs):
    if "nc" not in _CACHED:
        _CACHED["nc"] = build_program()
    nc = _CACHED["nc"]
    shared, per_core = _prep_inputs(**inputs)
    in_maps = [dict(shared, **pc) for pc in per_core]
    res = run_bass_kernel_spmd(nc, in_maps, list(range(B)))
    out = np.stack([res.results[i]["out"] for i in range(B)], axis=0)
    return out.astype(np.float32)


# revision 48
# speedup vs baseline: 1.1768x; 1.0912x over previous
"""Trainium2 Bass kernel for nn_CrossAttention (dense transformer block).

Sharding: data-parallel over batch — 8 batch elements, one per NeuronCore.
Each core runs the full block for its batch element:
  bias = Conv1x1(gelu(Conv1x1(log(attn_map[1:,1:] + eps))))
  MHA(q, kv) with bias added to scores; residual + LN; FFN; residual + LN.

v2: bf16 weights/activations on the matmul path, conv-bias pipeline with
batched scalar ops (log+exp share one ACT table set), head-pair packed
attention with DMA-xbar transposes, epilogues on vector/gpsimd.

Self-contained: hardcodes all shapes; host-side numpy prepares transposed /
packed weight layouts per core.
"""

import numpy as np
import ml_dtypes

import concourse.bass as bass
import concourse.mybir as mybir
import concourse.tile as tile
from concourse.tile import add_dep_helper
from concourse import bacc
from concourse.bass import ts
from concourse.bass_utils import run_bass_kernel_spmd
from concourse.masks import make_identity

AF = mybir.ActivationFunctionType
ALU = mybir.AluOpType
AX = mybir.AxisListType

B, S, D, H, DH, FF = 8, 512, 1024, 16, 64, 4096
CH, CHID = 16, 32
EPS_LOG = 1e-6
EPS_LN = 1e-6
P = 128
NQT = S // P          # 4 q-tiles
ND = D // P           # 8 d-blocks
NFF = FF // P         # 32 ff-blocks
AM = 513              # attn_map edge
NQI = 8               # q rows per conv group (8 groups of 16 partitions/hemi)

fp32 = mybir.dt.float32
bf16 = mybir.dt.bfloat16
fp8 = mybir.dt.float8e4
DR = mybir.MatmulPerfMode.DoubleRow

# fp8 e4m3 for both FFN matmuls (DoubleRow, 2 MACs/cell/cycle).
# Measured: pushes rel err to 2.8e-2 (> 2e-2 gate) — e4m3's 3 mantissa
# bits give ~2-3% rms weight error. Keep bf16.
FFN_FP8 = False

# 'dma' = xbar DMA transpose for attention probs, 'pe' = tensor-engine
ATT_TRANSPOSE = "dma"

_CACHED = {}


def _layernorm(nc, pool, out_ap, x_ap, gb, bb, eps_c):
    """out = (x - mean(x)) * rsqrt(var(x) + eps) * g + b over free dim (D)."""
    nsub = D // 512
    stats = pool.tile([P, nsub, nc.vector.BN_STATS_DIM], fp32, tag="ln_stats")
    for i in range(nsub):
        nc.vector.bn_stats(out=stats[:, i, :], in_=x_ap[:, ts(i, 512)])
    mv = pool.tile([P, nc.vector.BN_AGGR_DIM], fp32, tag="ln_mv")
    nc.vector.bn_aggr(out=mv, in_=stats)
    rstd = pool.tile([P, 1], fp32, tag="ln_rstd")
    nc.scalar.activation(rstd, mv[:, 1:2], AF.Sqrt, bias=eps_c, scale=1.0)
    nc.vector.reciprocal(out=rstd, in_=rstd)
    u = pool.tile([P, D], fp32, tag="ln_u")
    nc.vector.scalar_tensor_tensor(
        out=u, in0=x_ap, scalar=mv[:, 0:1], in1=gb,
        op0=ALU.subtract, op1=ALU.mult,
    )
    nc.vector.scalar_tensor_tensor(
        out=out_ap, in0=u, scalar=rstd[:, 0:1], in1=bb,
        op0=ALU.mult, op1=ALU.add,
    )


def _attention_qt(nc, qt, biasq, QtT, KtT, Vsb, ctxT, ident_b, p3sb, p3ps,
                  pcvps, exp_insts):
    """Attention for one 128-row q-tile, bias already staged in biasq."""
    den = p3sb.tile([P, H], fp32, tag="den")
    rec = p3sb.tile([P, H], fp32, tag="rec")
    for hp in range(8):  # 8 head pairs
        sc2 = p3ps.tile([P, 2, S], fp32, tag="sc2")
        for j in range(2):
            h = hp * 2 + j
            hb, ho = (h * DH) // P, (h * DH) % P
            nc.tensor.matmul(
                sc2[:, j, :],
                QtT[ho : ho + DH, hb, ts(qt, P)],
                KtT[ho : ho + DH, hb, :],
                start=True, stop=False,
            )
        for j in range(2):
            h = hp * 2 + j
            nc.tensor.matmul(
                sc2[:, j, :], ident_b, biasq[:, h, :],
                start=False, stop=True,
            )
        att2 = p3sb.tile([P, 2, S], bf16, tag="att2")
        exp_insts.append(nc.scalar.activation(
            att2.rearrange("p a b -> p (a b)"),
            sc2.rearrange("p a b -> p (a b)"),
            AF.Exp,
        ))
        nc.vector.tensor_reduce(
            out=den[:, hp * 2 : hp * 2 + 2], in_=att2,
            axis=AX.X, op=ALU.add,
        )
        nc.vector.reciprocal(
            out=rec[:, hp * 2 : hp * 2 + 2],
            in_=den[:, hp * 2 : hp * 2 + 2],
        )
        cx = p3ps.tile([P, P], fp32, tag="cx")
        for j in range(2):
            h = hp * 2 + j
            attn = p3sb.tile([P, S], bf16, tag="attn", bufs=3)
            nc.vector.tensor_scalar_mul(
                attn, att2[:, j, :], rec[:, h : h + 1]
            )
            atTs = p3sb.tile([P, NQT, P], bf16, tag="atTs", bufs=3)
            if ATT_TRANSPOSE == "dma":
                nc.sync.dma_start_transpose(atTs, attn)
            else:
                atT_ps = p3ps.tile([P, S], bf16, tag="atT")
                for kt in range(NQT):
                    nc.tensor.transpose(
                        atT_ps[:, ts(kt, P)], attn[:, ts(kt, P)], ident_b
                    )
                nc.vector.tensor_copy(
                    atTs.rearrange("p a b -> p (a b)"), atT_ps
                )
            for kt in range(NQT):
                nc.tensor.matmul(
                    cx[j * DH : (j + 1) * DH, :],
                    Vsb[:, kt, h * DH : (h + 1) * DH],
                    atTs[:, kt, :],
                    start=(kt == 0), stop=(kt == NQT - 1),
                    tile_position=(0, j * DH),
                )
        nc.vector.tensor_copy(ctxT[:, hp, ts(qt, P)], cx)


def build_program():
    nc = bacc.Bacc(None)

    # ---------------- DRAM I/O ----------------
    qT_e = nc.dram_tensor("qT", [D, S], bf16, kind="ExternalInput")
    kvT_e = nc.dram_tensor("kvT", [D, S], bf16, kind="ExternalInput")
    qin_e = nc.dram_tensor("qin", [S, D], fp32, kind="ExternalInput")
    amap_e = nc.dram_tensor("amap", [CH, AM, AM], fp32, kind="ExternalInput")
    wqT_e = nc.dram_tensor("wqT", [D, D], bf16, kind="ExternalInput")
    wkT_e = nc.dram_tensor("wkT", [D, D], bf16, kind="ExternalInput")
    wvT_e = nc.dram_tensor("wvT", [D, D], bf16, kind="ExternalInput")
    wmT_e = nc.dram_tensor("wmT", [D, D], bf16, kind="ExternalInput")
    fdt = fp8 if FFN_FP8 else bf16
    # FFN1 weights: [ffb, p, (dpair b, j, f)] = Wf1.T[(2b+j)*128+p, ffb*128+f]
    wf1P_e = nc.dram_tensor("wf1P", [NFF, P, D], fdt, kind="ExternalInput")
    # FFN2 weights: [fb, p, (j, d)] = Wf2.T[(2fb+j)*128+p, d]
    wf2P_e = nc.dram_tensor("wf2P", [NFF // 2, P, 2 * D], fdt, kind="ExternalInput")
    c1A_e = nc.dram_tensor("c1A", [P, P], bf16, kind="ExternalInput")
    c1B_e = nc.dram_tensor("c1B", [P, P], bf16, kind="ExternalInput")
    c2A_e = nc.dram_tensor("c2A", [P, P], bf16, kind="ExternalInput")
    c2B_e = nc.dram_tensor("c2B", [P, P], bf16, kind="ExternalInput")
    # per-partition bias columns
    bqc_e = nc.dram_tensor("bqc", [P, ND], fp32, kind="ExternalInput")   # bq/8
    bkc_e = nc.dram_tensor("bkc", [P, ND], fp32, kind="ExternalInput")
    bc1A_e = nc.dram_tensor("bc1A", [P, 1], fp32, kind="ExternalInput")
    bc1B_e = nc.dram_tensor("bc1B", [P, 1], fp32, kind="ExternalInput")
    bc2c_e = nc.dram_tensor("bc2c", [P, 1], fp32, kind="ExternalInput")
    bf1c_e = nc.dram_tensor("bf1c", [P, NFF], fp32, kind="ExternalInput")
    # bias rows (K=1 matmul trick)
    bvr_e = nc.dram_tensor("bvr", [1, D], bf16, kind="ExternalInput")
    bmr_e = nc.dram_tensor("bmr", [1, D], bf16, kind="ExternalInput")
    bf2r_e = nc.dram_tensor("bf2r", [1, D], bf16, kind="ExternalInput")
    onesb_e = nc.dram_tensor("onesb", [1, P], bf16, kind="ExternalInput")
    # LN params as rows
    g1r_e = nc.dram_tensor("g1r", [1, D], bf16, kind="ExternalInput")
    b1r_e = nc.dram_tensor("b1r", [1, D], bf16, kind="ExternalInput")
    g2r_e = nc.dram_tensor("g2r", [1, D], bf16, kind="ExternalInput")
    b2r_e = nc.dram_tensor("b2r", [1, D], bf16, kind="ExternalInput")

    out_e = nc.dram_tensor("out", [S, D], fp32, kind="ExternalOutput")

    with tile.TileContext(nc) as tc:
        # ------------- persistent pools -------------
        const_cm = tc.tile_pool(name="const", bufs=1)
        const = const_cm.__enter__()
        dram_cm = tc.tile_pool(name="dstage", bufs=1, space="DRAM")
        dram = dram_cm.__enter__()
        bstage = dram.tile([S, H, S], bf16)
        resid_cm = tc.tile_pool(name="resid", bufs=1)  # ctxT/xln (ph3-7)
        resid = resid_cm.__enter__()
        bigE_cm = tc.tile_pool(name="bigE", bufs=1)   # Qt/Kt/V (ph1-3)
        bigE = bigE_cm.__enter__()

        ident_b = const.tile([P, P], bf16)
        make_identity(nc, ident_b)
        ident_f = const.tile([P, P], fp32)
        make_identity(nc, ident_f)

        eps_log_c = const.tile([P, 1], fp32)
        nc.vector.memset(eps_log_c, EPS_LOG)
        eps_ln_c = const.tile([P, 1], fp32)
        nc.vector.memset(eps_ln_c, EPS_LN)

        c1A = const.tile([P, P], bf16)
        c1B = const.tile([P, P], bf16)
        c2A = const.tile([P, P], bf16)
        c2B = const.tile([P, P], bf16)
        nc.scalar.dma_start(out=c1A, in_=c1A_e[:, :])
        nc.scalar.dma_start(out=c1B, in_=c1B_e[:, :])
        nc.scalar.dma_start(out=c2A, in_=c2A_e[:, :])
        nc.scalar.dma_start(out=c2B, in_=c2B_e[:, :])
        bc1A = const.tile([P, 1], fp32)
        bc1B = const.tile([P, 1], fp32)
        bc2c = const.tile([P, 1], fp32)
        nc.scalar.dma_start(out=bc1A, in_=bc1A_e[:, :])
        nc.scalar.dma_start(out=bc1B, in_=bc1B_e[:, :])
        nc.scalar.dma_start(out=bc2c, in_=bc2c_e[:, :])
        bqc = const.tile([P, ND], fp32)
        bkc = const.tile([P, ND], fp32)
        bf1c = const.tile([P, NFF], fp32)
        nc.scalar.dma_start(out=bqc, in_=bqc_e[:, :])
        nc.scalar.dma_start(out=bkc, in_=bkc_e[:, :])
        nc.scalar.dma_start(out=bf1c, in_=bf1c_e[:, :])
        bvr = const.tile([1, D], bf16)
        bmr = const.tile([1, D], bf16)
        bf2r = const.tile([1, D], bf16)
        onesb = const.tile([1, P], bf16)
        nc.scalar.dma_start(out=bvr, in_=bvr_e[:, :])
        nc.scalar.dma_start(out=bmr, in_=bmr_e[:, :])
        nc.scalar.dma_start(out=bf2r, in_=bf2r_e[:, :])
        nc.scalar.dma_start(out=onesb, in_=onesb_e[:, :])

        # LN param broadcast tiles [128, D]
        g1b = const.tile([P, D], bf16)
        b1b = const.tile([P, D], bf16)
        g2b = const.tile([P, D], bf16)
        b2b = const.tile([P, D], bf16)
        for dst, src_e in ((g1b, g1r_e), (b1b, b1r_e), (g2b, g2r_e), (b2b, b2r_e)):
            row = const.tile([1, D], bf16, tag="lnrow", name="lnrow")
            nc.scalar.dma_start(out=row, in_=src_e[:, :])
            nc.gpsimd.partition_broadcast(dst, row[0:1, :])

        # attention-phase residents
        QtT = bigE.tile([P, ND, S], bf16)      # [o-part, o-blk, s]  ((Wq x + bq)/8)
        KtT = bigE.tile([P, ND, S], bf16)
        Vsb = bigE.tile([P, NQT, D], bf16)     # [k-part, k-blk, o]
        ctxT = resid.tile([P, ND, S], bf16)    # [(h,dh)-part, blk, q]
        xln = resid.tile([P, NQT, D], fp32)    # LN1 out [s-part, s-blk, d]

        # =========== Phase 1: projections ===========
        with (
            tc.tile_pool(name="p1x", bufs=1) as p1x,
            tc.tile_pool(name="p1w", bufs=3) as p1w,
            tc.tile_pool(name="p1ps", bufs=1, space="PSUM") as p1ps,
        ):
            qT = p1x.tile([P, ND, S], bf16)
            kvT = p1x.tile([P, ND, S], bf16)
            for dblk in range(ND):
                nc.sync.dma_start(
                    out=qT[:, dblk, :],
                    in_=qT_e[dblk * P : (dblk + 1) * P, :],
                )
                nc.sync.dma_start(
                    out=kvT[:, dblk, :],
                    in_=kvT_e[dblk * P : (dblk + 1) * P, :],
                )

            for wsrc, xsb, dst, bcol in (
                (wqT_e, qT, QtT, bqc),
                (wkT_e, kvT, KtT, bkc),
            ):
                psums = [p1ps.tile([P, S], fp32, tag=f"pp{i}", name=f"pp{i}") for i in range(ND)]
                for dblk in range(ND):
                    wch = p1w.tile([P, D], bf16, tag="wch")
                    nc.sync.dma_start(
                        out=wch, in_=wsrc[dblk * P : (dblk + 1) * P, :]
                    )
                    for ob in range(ND):
                        nc.tensor.matmul(
                            psums[ob],
                            wch[:, ts(ob, P)],
                            xsb[:, dblk, :],
                            start=(dblk == 0),
                            stop=(dblk == ND - 1),
                        )
                for ob in range(ND):
                    nc.vector.tensor_scalar_add(
                        dst[:, ob, :], psums[ob], bcol[:, ob : ob + 1]
                    )

            vps = [
                [p1ps.tile([P, S], fp32, tag=f"pp{st * 2 + oh}", name=f"vp{st}{oh}") for oh in range(2)]
                for st in range(NQT)
            ]
            for st in range(NQT):
                for oh in range(2):
                    nc.tensor.matmul(
                        vps[st][oh], onesb, bvr[:, ts(oh, S)],
                        start=True, stop=False,
                    )
            for dblk in range(ND):
                wch = p1w.tile([P, D], bf16, tag="wch")
                nc.sync.dma_start(out=wch, in_=wvT_e[dblk * P : (dblk + 1) * P, :])
                for st in range(NQT):
                    for oh in range(2):
                        nc.tensor.matmul(
                            vps[st][oh],
                            kvT[:, dblk, ts(st, P)],
                            wch[:, ts(oh, S)],
                            start=False,
                            stop=(dblk == ND - 1),
                        )
            for st in range(NQT):
                for oh in range(2):
                    nc.vector.tensor_copy(Vsb[:, st, ts(oh, S)], vps[st][oh])

        # =========== Phase 2+3: conv bias + attention, per qtile ===========
        with (
            tc.tile_pool(name="pbias", bufs=2) as pbias,
            tc.tile_pool(name="pcv", bufs=2) as pcv,
            tc.tile_pool(name="pcvps", bufs=1, space="PSUM") as pcvps,
            tc.tile_pool(name="pc2ps", bufs=1, space="PSUM") as pc2ps,
            tc.tile_pool(name="p3sb", bufs=2) as p3sb,
            tc.tile_pool(name="p3ps", bufs=1, space="PSUM") as p3ps,
        ):
            prev_last_exp = None
            for blk in range(NQT // 2):
                # ---- conv bias for qtiles 2*blk, 2*blk+1 ----
                logms = {}
                first_log = None
                last_gelu = None
                for sub in range(2):
                    qt = blk * 2 + sub
                    for hemi in range(2):
                        qbase = qt * P + hemi * 64
                        amt = pcv.tile([P, NQI, S], fp32, tag="amt")
                        for g in range(8):
                            src = bass.AP(
                                tensor=amap_e,
                                offset=(1 + qbase + NQI * g) * AM + 1,
                                ap=[[AM * AM, CH], [AM, NQI], [1, S]],
                            )
                            nc.sync.dma_start(out=amt[CH * g : CH * (g + 1)], in_=src)
                        logm = pcv.tile([P, NQI * S], bf16, tag="logm", bufs=4)
                        li = nc.scalar.activation(
                            logm, amt.rearrange("p a b -> p (a b)"), AF.Ln,
                            bias=eps_log_c, scale=1.0,
                        )
                        if first_log is None:
                            first_log = li
                        logms[(sub, hemi)] = logm
                # keep the ACT table from ping-ponging between the Ln/Exp set
                # and the Gelu set: this block's logs come after the previous
                # block's last softmax exp
                if prev_last_exp is not None and first_log is not None:
                    add_dep_helper(first_log.ins, prev_last_exp.ins, False,
                                   "act-table grouping")
                for sub in range(2):
                    qt = blk * 2 + sub
                    for hemi in range(2):
                        qbase = qt * P + hemi * 64
                        logm = logms[(sub, hemi)]
                        c2sb = pcv.tile([P, NQI, S], bf16, tag="c2sb", bufs=2)
                        for ch in range(NQI):
                            pAB = pcvps.tile([P, 2, S], fp32, tag="pAB", bufs=2)
                            nc.tensor.matmul(
                                pAB[:, 0, :], c1A, logm[:, ts(ch, S)],
                                start=True, stop=True,
                            )
                            nc.tensor.matmul(
                                pAB[:, 1, :], c1B, logm[:, ts(ch, S)],
                                start=True, stop=True,
                            )
                            gAB = pcv.tile([P, 2, S], bf16, tag="gAB", bufs=2)
                            nc.scalar.activation(
                                gAB[:, 0, :], pAB[:, 0, :], AF.Gelu,
                                bias=bc1A, scale=1.0,
                            )
                            last_gelu = nc.scalar.activation(
                                gAB[:, 1, :], pAB[:, 1, :], AF.Gelu,
                                bias=bc1B, scale=1.0,
                            )
                            pC = pc2ps.tile([P, S], fp32, tag="pC")
                            nc.tensor.matmul(
                                pC, c2A, gAB[:, 0, :], start=True, stop=False
                            )
                            nc.tensor.matmul(
                                pC, c2B, gAB[:, 1, :], start=False, stop=True
                            )
                            nc.vector.tensor_scalar_add(c2sb[:, ch, :], pC, bc2c)
                        # scatter to DRAM bias stage in [q, h, k] order
                        for g in range(8):
                            q0 = qbase + NQI * g
                            nc.gpsimd.dma_start(
                                out=bstage[q0 : q0 + NQI].rearrange("i h k -> h i k"),
                                in_=c2sb[CH * g : CH * (g + 1)],
                            )

                exp_insts = []
                for sub in range(2):
                    qt = blk * 2 + sub
                    biasq = pbias.tile([P, H, S], bf16, tag="biasq")
                    nc.sync.dma_start(
                        out=biasq.rearrange("p h k -> p (h k)"),
                        in_=bstage[qt * P : (qt + 1) * P].rearrange("q h k -> q (h k)"),
                    )
                    _attention_qt(nc, qt, biasq, QtT, KtT, Vsb, ctxT,
                                  ident_b, p3sb, p3ps, pcvps, exp_insts)
                # this block's exps come after this block's last gelu
                if last_gelu is not None and exp_insts:
                    add_dep_helper(exp_insts[0].ins, last_gelu.ins, False,
                                   "act-table grouping")
                prev_last_exp = exp_insts[-1] if exp_insts else None

        bigE_cm.__exit__(None, None, None)

        # =========== Phase 4: merge + residual + LN1 ===========
        with (
            tc.tile_pool(name="p4sb", bufs=3) as p4sb,
            tc.tile_pool(name="p4ps", bufs=1, space="PSUM") as p4ps,
        ):
            wmr = p4sb.tile([P, ND, D], bf16, tag="wmr", bufs=1)
            for dblk in range(ND):
                nc.sync.dma_start(
                    out=wmr[:, dblk, :], in_=wmT_e[dblk * P : (dblk + 1) * P, :]
                )
            for st in range(NQT):
                mps = p4ps.tile([P, 2, S], fp32, tag="mps", bufs=2)
                for oh in range(2):
                    nc.tensor.matmul(
                        mps[:, oh, :], onesb, bmr[:, ts(oh, S)],
                        start=True, stop=False,
                    )
                for dblk in range(ND):
                    for oh in range(2):
                        nc.tensor.matmul(
                            mps[:, oh, :],
                            ctxT[:, dblk, ts(st, P)],
                            wmr[:, dblk, ts(oh, S)],
                            start=False,
                            stop=(dblk == ND - 1),
                        )
                qtile = p4sb.tile([P, D], fp32, tag="qtile")
                nc.sync.dma_start(out=qtile, in_=qin_e[st * P : (st + 1) * P, :])
                x1 = p4sb.tile([P, D], fp32, tag="x1")
                for oh in range(2):
                    nc.vector.tensor_tensor(
                        out=x1[:, ts(oh, S)], in0=mps[:, oh, :],
                        in1=qtile[:, ts(oh, S)], op=ALU.add,
                    )
                _layernorm(nc, p4sb, xln[:, st, :], x1, g1b, b1b, eps_ln_c)

        bigL_cm = tc.tile_pool(name="bigL", bufs=1)
        bigL = bigL_cm.__enter__()
        fdt_ = fp8 if FFN_FP8 else bf16
        xlnT = bigL.tile([P, ND, S], fdt_)
        y1T = bigL.tile([P, NFF, S], fdt_)
        wf2r = bigL.tile([P, NFF // 2, 2 * D], fdt_)
        for fb in range(NFF // 2):
            nc.sync.dma_start(out=wf2r[:, fb, :], in_=wf2P_e[fb])

        # =========== Phase 5: transpose x_ln ===========
        with tc.tile_pool(name="p5ps", bufs=2, space="PSUM") as p5ps:
            for dblk in range(ND):
                tp = p5ps.tile([P, S], fp32, tag="tp")
                for st in range(NQT):
                    nc.tensor.transpose(
                        tp[:, ts(st, P)], xln[:, st, ts(dblk, P)], ident_f
                    )
                nc.vector.tensor_copy(xlnT[:, dblk, :], tp)

        bigL_cm = tc.tile_pool(name="bigL", bufs=1)
        bigL = bigL_cm.__enter__()
        fdt_ = fp8 if FFN_FP8 else bf16
        xlnT = bigL.tile([P, ND, S], bf16)
        y1T = bigL.tile([P, NFF, S], fdt_)
        wf2r = bigL.tile([P, NFF // 2, 2 * D], fdt_)
        for fb in range(NFF // 2):
            nc.sync.dma_start(out=wf2r[:, fb, :], in_=wf2P_e[fb])

        # =========== Phase 5: transpose x_ln ===========
        with tc.tile_pool(name="p5ps", bufs=2, space="PSUM") as p5ps:
            for dblk in range(ND):
                tp = p5ps.tile([P, S], fp32, tag="tp")
                for st in range(NQT):
                    nc.tensor.transpose(
                        tp[:, ts(st, P)], xln[:, st, ts(dblk, P)], ident_f
                    )
                nc.vector.tensor_copy(xlnT[:, dblk, :], tp)

        # =========== Phase 6: FFN1 + relu ===========
        with (
            tc.tile_pool(name="p6w", bufs=4) as p6w,
            tc.tile_pool(name="p6ps", bufs=2, space="PSUM") as p6ps,
        ):
            for ffb in range(NFF):
                fps = p6ps.tile([P, S], fp32, tag="fps")
                if FFN_FP8:
                    wf1c = p6w.tile([P, ND // 2, 2, P], fp8, tag="wf1c")
                    nc.sync.dma_start(
                        out=wf1c.rearrange("p a b c -> p (a b c)"), in_=wf1P_e[ffb]
                    )
                    for b in range(ND // 2):
                        nc.tensor.matmul(
                            fps,
                            wf1c[:, b, :, :],
                            xlnT[:, 2 * b : 2 * b + 2, :],
                            start=(b == 0), stop=(b == ND // 2 - 1),
                            perf_mode=DR,
                        )
                else:
                    wf1c = p6w.tile([P, ND, P], bf16, tag="wf1c")
                    nc.sync.dma_start(
                        out=wf1c.rearrange("p a b -> p (a b)"), in_=wf1P_e[ffb]
                    )
                    for dblk in range(ND):
                        nc.tensor.matmul(
                            fps,
                            wf1c[:, dblk, :],
                            xlnT[:, dblk, :],
                            start=(dblk == 0), stop=(dblk == ND - 1),
                        )
                nc.vector.tensor_scalar(
                    out=y1T[:, ffb, :], in0=fps,
                    scalar1=bf1c[:, ffb : ffb + 1], scalar2=0.0,
                    op0=ALU.add, op1=ALU.max,
                )

        # =========== Phase 7: FFN2 + residual + LN2 + out ===========
        with (
            tc.tile_pool(name="p7sb", bufs=2) as p7sb,
            tc.tile_pool(name="p7ps", bufs=1, space="PSUM") as p7ps,
        ):
            for st in range(NQT):
                fps2 = p7ps.tile([P, 2, S], fp32, tag="fps2", bufs=2)
                for oh in range(2):
                    nc.tensor.matmul(
                        fps2[:, oh, :], onesb, bf2r[:, ts(oh, S)],
                        start=True, stop=False,
                    )
                for ffb in range(NFF):
                    wch = wf2r[:, ffb // 2, (ffb % 2) * D : (ffb % 2 + 1) * D]
                    for oh in range(2):
                        nc.tensor.matmul(
                            fps2[:, oh, :],
                            y1T[:, ffb, ts(st, P)],
                            wch[:, ts(oh, S)],
                            start=False,
                            stop=(ffb == NFF - 1),
                        )
                x2 = p7sb.tile([P, D], fp32, tag="x2")
                for oh in range(2):
                    nc.vector.tensor_tensor(
                        out=x2[:, ts(oh, S)], in0=fps2[:, oh, :],
                        in1=xln[:, st, ts(oh, S)], op=ALU.add,
                    )
                xout = p7sb.tile([P, D], fp32, tag="xout")
                _layernorm(nc, p7sb, xout, x2, g2b, b2b, eps_ln_c)
                nc.sync.dma_start(out=out_e[st * P : (st + 1) * P, :], in_=xout)

        bigL_cm.__exit__(None, None, None)
        resid_cm.__exit__(None, None, None)
        dram_cm.__exit__(None, None, None)
        const_cm.__exit__(None, None, None)

    nc.finalize()
    return nc


def _prep_inputs(q, kv, attn_map, Wq, bq, Wk, bk, Wv, bv, Wm, bm,
                 Wc1, bc1, Wc2, bc2, Wf1, bf1, Wf2, bf2, g1, b1, g2, b2):
    """Host-side packing. Returns (shared dict, per-core list of dicts)."""
    f32 = np.float32
    bf = ml_dtypes.bfloat16

    def c(a):
        return np.ascontiguousarray(np.asarray(a), dtype=f32)

    def cb(a):
        return np.ascontiguousarray(np.asarray(a, dtype=f32)).astype(bf)

    Wq, Wk, Wv, Wm = c(Wq), c(Wk), c(Wv), c(Wm)
    Wc1, Wc2, Wf1, Wf2 = c(Wc1), c(Wc2), c(Wf1), c(Wf2)
    bq, bk, bv, bm = c(bq), c(bk), c(bv), c(bm)
    bc1, bc2, bf1, bf2 = c(bc1), c(bc2), c(bf1), c(bf2)
    g1, b1, g2, b2 = c(g1), c(b1), c(g2), c(b2)

    fdt = ml_dtypes.float8_e4m3 if FFN_FP8 else bf

    # FFN1 weights packed so each SBUF partition gets a contiguous row.
    # fp8/DoubleRow: wf1P[ffb, p, ((b,j),f)] = Wf1.T[(2b+j)*128+p, ffb*128+f]
    # bf16:          wf1P[ffb, p, (n,f)]     = Wf1.T[n*128+p, ffb*128+f]
    Wf1T = np.ascontiguousarray(Wf1.T)            # [D, FF]
    wf1P = np.ascontiguousarray(
        Wf1T.reshape(ND, P, NFF, P).transpose(2, 1, 0, 3).reshape(NFF, P, D)
    ).astype(fdt)
    # FFN2 weights: wf2P[fb, p, (j,d)] = Wf2.T[(2fb+j)*128+p, d]
    Wf2T = np.ascontiguousarray(Wf2.T)            # [FF, D]
    wf2P = np.ascontiguousarray(
        Wf2T.reshape(NFF // 2, 2, P, D).transpose(0, 2, 1, 3).reshape(NFF // 2, P, 2 * D)
    ).astype(fdt)

    shared = {
        "wqT": cb(Wq.T * 0.125), "wkT": cb(Wk.T), "wvT": cb(Wv.T),
        "wmT": cb(Wm.T),
        "wf1P": wf1P,
        "wf2P": wf2P,
        "bqc": c((bq / 8.0).reshape(ND, P).T),
        "bkc": c(bk.reshape(ND, P).T),
        "bf1c": c(bf1.reshape(NFF, P).T),
        "bvr": cb(bv.reshape(1, D)), "bmr": cb(bm.reshape(1, D)),
        "bf2r": cb(bf2.reshape(1, D)),
        "onesb": np.ones((1, P), bf),
        "g1r": cb(g1.reshape(1, D)), "b1r": cb(b1.reshape(1, D)),
        "g2r": cb(g2.reshape(1, D)), "b2r": cb(b2.reshape(1, D)),
    }
    # conv block-diag lhsT [K, M]: out[(g,oh)] = sum_c lhsT[(g,c),(g,oh)] rhs[(g,c)]
    c1A = np.zeros((P, P), f32)
    c1B = np.zeros((P, P), f32)
    c2A = np.zeros((P, P), f32)
    c2B = np.zeros((P, P), f32)
    for g in range(8):
        sl = slice(g * 16, g * 16 + 16)
        c1A[sl, sl] = Wc1[0:16, :].T     # [c, oh]
        c1B[sl, sl] = Wc1[16:32, :].T
        c2A[sl, sl] = Wc2[:, 0:16].T     # [ci, h]
        c2B[sl, sl] = Wc2[:, 16:32].T
    shared["c1A"] = c1A.astype(bf)
    shared["c1B"] = c1B.astype(bf)
    shared["c2A"] = c2A.astype(bf)
    shared["c2B"] = c2B.astype(bf)
    shared["bc1A"] = np.tile(bc1[0:16], 8).reshape(P, 1).astype(f32)
    shared["bc1B"] = np.tile(bc1[16:32], 8).reshape(P, 1).astype(f32)
    shared["bc2c"] = np.tile(bc2, 8).reshape(P, 1).astype(f32)

    q = c(q)
    kv = c(kv)
    attn_map = np.asarray(attn_map)
    per_core = []
    for b in range(B):
        per_core.append({
            "qT": np.ascontiguousarray(q[b].T).astype(bf),
            "kvT": np.ascontiguousarray(kv[b].T).astype(bf),
            "qin": q[b],
            "amap": c(attn_map[b]),
        })
    return shared, per_core


def kernel(**inputs):
    if "nc" not in _CACHED:
        _CACHED["nc"] = build_program()
    nc = _CACHED["nc"]
    shared, per_core = _prep_inputs(**inputs)
    in_maps = [dict(shared, **pc) for pc in per_core]
    res = run_bass_kernel_spmd(nc, in_maps, list(range(B)))
    out = np.stack([res.results[i]["out"] for i in range(B)], axis=0)
    return out.astype(np.float32)


# revision 49
# speedup vs baseline: 1.1964x; 1.0166x over previous
"""Trainium2 Bass kernel for nn_CrossAttention (dense transformer block).

Sharding: data-parallel over batch — 8 batch elements, one per NeuronCore.
Each core runs the full block for its batch element:
  bias = Conv1x1(gelu(Conv1x1(log(attn_map[1:,1:] + eps))))
  MHA(q, kv) with bias added to scores; residual + LN; FFN; residual + LN.

v2: bf16 weights/activations on the matmul path, conv-bias pipeline with
batched scalar ops (log+exp share one ACT table set), head-pair packed
attention with DMA-xbar transposes, epilogues on vector/gpsimd.

Self-contained: hardcodes all shapes; host-side numpy prepares transposed /
packed weight layouts per core.
"""

import numpy as np
import ml_dtypes

import concourse.bass as bass
import concourse.mybir as mybir
import concourse.tile as tile
from concourse.tile import add_dep_helper
from concourse import bacc
from concourse.bass import ts
from concourse.bass_utils import run_bass_kernel_spmd
from concourse.masks import make_identity

AF = mybir.ActivationFunctionType
ALU = mybir.AluOpType
AX = mybir.AxisListType

B, S, D, H, DH, FF = 8, 512, 1024, 16, 64, 4096
CH, CHID = 16, 32
EPS_LOG = 1e-6
EPS_LN = 1e-6
P = 128
NQT = S // P          # 4 q-tiles
ND = D // P           # 8 d-blocks
NFF = FF // P         # 32 ff-blocks
AM = 513              # attn_map edge
NQI = 8               # q rows per conv group (8 groups of 16 partitions/hemi)

fp32 = mybir.dt.float32
bf16 = mybir.dt.bfloat16
fp8 = mybir.dt.float8e4
DR = mybir.MatmulPerfMode.DoubleRow

# fp8 e4m3 for both FFN matmuls (DoubleRow, 2 MACs/cell/cycle).
# Measured: pushes rel err to 2.8e-2 (> 2e-2 gate) — e4m3's 3 mantissa
# bits give ~2-3% rms weight error. Keep bf16.
FFN_FP8 = False

# 'dma' = xbar DMA transpose for attention probs, 'pe' = tensor-engine
ATT_TRANSPOSE = "dma"

_CACHED = {}


def _layernorm(nc, pool, out_ap, x_ap, gb, bb, eps_c):
    """out = (x - mean(x)) * rsqrt(var(x) + eps) * g + b over free dim (D)."""
    nsub = D // 512
    stats = pool.tile([P, nsub, nc.vector.BN_STATS_DIM], fp32, tag="ln_stats")
    for i in range(nsub):
        nc.vector.bn_stats(out=stats[:, i, :], in_=x_ap[:, ts(i, 512)])
    mv = pool.tile([P, nc.vector.BN_AGGR_DIM], fp32, tag="ln_mv")
    nc.vector.bn_aggr(out=mv, in_=stats)
    rstd = pool.tile([P, 1], fp32, tag="ln_rstd")
    nc.scalar.activation(rstd, mv[:, 1:2], AF.Sqrt, bias=eps_c, scale=1.0)
    nc.vector.reciprocal(out=rstd, in_=rstd)
    u = pool.tile([P, D], fp32, tag="ln_u")
    nc.vector.scalar_tensor_tensor(
        out=u, in0=x_ap, scalar=mv[:, 0:1], in1=gb,
        op0=ALU.subtract, op1=ALU.mult,
    )
    nc.vector.scalar_tensor_tensor(
        out=out_ap, in0=u, scalar=rstd[:, 0:1], in1=bb,
        op0=ALU.mult, op1=ALU.add,
    )


def _attention_qt(nc, qt, biasq, QtT, KtT, Vsb, ctxT, ident_b, p3sb, p3ps,
                  pcvps, exp_insts):
    """Attention for one 128-row q-tile, bias already staged in biasq."""
    den = p3sb.tile([P, H], fp32, tag="den")
    rec = p3sb.tile([P, H], fp32, tag="rec")
    for hp in range(8):  # 8 head pairs
        sc2 = p3ps.tile([P, 2, S], fp32, tag="sc2")
        for j in range(2):
            h = hp * 2 + j
            hb, ho = (h * DH) // P, (h * DH) % P
            nc.tensor.matmul(
                sc2[:, j, :],
                QtT[ho : ho + DH, hb, ts(qt, P)],
                KtT[ho : ho + DH, hb, :],
                start=True, stop=False,
            )
        for j in range(2):
            h = hp * 2 + j
            nc.tensor.matmul(
                sc2[:, j, :], ident_b, biasq[:, h, :],
                start=False, stop=True,
            )
        att2 = p3sb.tile([P, 2, S], bf16, tag="att2")
        exp_insts.append(nc.scalar.activation(
            att2.rearrange("p a b -> p (a b)"),
            sc2.rearrange("p a b -> p (a b)"),
            AF.Exp,
        ))
        nc.vector.tensor_reduce(
            out=den[:, hp * 2 : hp * 2 + 2], in_=att2,
            axis=AX.X, op=ALU.add,
        )
        nc.vector.reciprocal(
            out=rec[:, hp * 2 : hp * 2 + 2],
            in_=den[:, hp * 2 : hp * 2 + 2],
        )
        cx = p3ps.tile([P, P], fp32, tag="cx")
        for j in range(2):
            h = hp * 2 + j
            attn = p3sb.tile([P, S], bf16, tag="attn", bufs=3)
            nc.vector.tensor_scalar_mul(
                attn, att2[:, j, :], rec[:, h : h + 1]
            )
            atTs = p3sb.tile([P, NQT, P], bf16, tag="atTs", bufs=3)
            if ATT_TRANSPOSE == "dma":
                eng = nc.sync if j == 0 else nc.scalar
                eng.dma_start_transpose(atTs, attn)
            else:
                atT_ps = p3ps.tile([P, S], bf16, tag="atT")
                for kt in range(NQT):
                    nc.tensor.transpose(
                        atT_ps[:, ts(kt, P)], attn[:, ts(kt, P)], ident_b
                    )
                nc.vector.tensor_copy(
                    atTs.rearrange("p a b -> p (a b)"), atT_ps
                )
            for kt in range(NQT):
                nc.tensor.matmul(
                    cx[j * DH : (j + 1) * DH, :],
                    Vsb[:, kt, h * DH : (h + 1) * DH],
                    atTs[:, kt, :],
                    start=(kt == 0), stop=(kt == NQT - 1),
                    tile_position=(0, j * DH),
                )
        nc.vector.tensor_copy(ctxT[:, hp, ts(qt, P)], cx)


def build_program():
    nc = bacc.Bacc(None)

    # ---------------- DRAM I/O ----------------
    qT_e = nc.dram_tensor("qT", [D, S], bf16, kind="ExternalInput")
    kvT_e = nc.dram_tensor("kvT", [D, S], bf16, kind="ExternalInput")
    qin_e = nc.dram_tensor("qin", [S, D], fp32, kind="ExternalInput")
    amap_e = nc.dram_tensor("amap", [CH, AM, AM], fp32, kind="ExternalInput")
    wqT_e = nc.dram_tensor("wqT", [D, D], bf16, kind="ExternalInput")
    wkT_e = nc.dram_tensor("wkT", [D, D], bf16, kind="ExternalInput")
    wvT_e = nc.dram_tensor("wvT", [D, D], bf16, kind="ExternalInput")
    wmT_e = nc.dram_tensor("wmT", [D, D], bf16, kind="ExternalInput")
    fdt = fp8 if FFN_FP8 else bf16
    # FFN1 weights: [ffb, p, (dpair b, j, f)] = Wf1.T[(2b+j)*128+p, ffb*128+f]
    wf1P_e = nc.dram_tensor("wf1P", [NFF, P, D], fdt, kind="ExternalInput")
    # FFN2 weights: [fb, p, (j, d)] = Wf2.T[(2fb+j)*128+p, d]
    wf2P_e = nc.dram_tensor("wf2P", [NFF // 2, P, 2 * D], fdt, kind="ExternalInput")
    c1A_e = nc.dram_tensor("c1A", [P, P], bf16, kind="ExternalInput")
    c1B_e = nc.dram_tensor("c1B", [P, P], bf16, kind="ExternalInput")
    c2A_e = nc.dram_tensor("c2A", [P, P], bf16, kind="ExternalInput")
    c2B_e = nc.dram_tensor("c2B", [P, P], bf16, kind="ExternalInput")
    # per-partition bias columns
    bqc_e = nc.dram_tensor("bqc", [P, ND], fp32, kind="ExternalInput")   # bq/8
    bkc_e = nc.dram_tensor("bkc", [P, ND], fp32, kind="ExternalInput")
    bc1A_e = nc.dram_tensor("bc1A", [P, 1], fp32, kind="ExternalInput")
    bc1B_e = nc.dram_tensor("bc1B", [P, 1], fp32, kind="ExternalInput")
    bc2c_e = nc.dram_tensor("bc2c", [P, 1], fp32, kind="ExternalInput")
    bf1c_e = nc.dram_tensor("bf1c", [P, NFF], fp32, kind="ExternalInput")
    # bias rows (K=1 matmul trick)
    bvr_e = nc.dram_tensor("bvr", [1, D], bf16, kind="ExternalInput")
    bmr_e = nc.dram_tensor("bmr", [1, D], bf16, kind="ExternalInput")
    bf2r_e = nc.dram_tensor("bf2r", [1, D], bf16, kind="ExternalInput")
    onesb_e = nc.dram_tensor("onesb", [1, P], bf16, kind="ExternalInput")
    # LN params as rows
    g1r_e = nc.dram_tensor("g1r", [1, D], bf16, kind="ExternalInput")
    b1r_e = nc.dram_tensor("b1r", [1, D], bf16, kind="ExternalInput")
    g2r_e = nc.dram_tensor("g2r", [1, D], bf16, kind="ExternalInput")
    b2r_e = nc.dram_tensor("b2r", [1, D], bf16, kind="ExternalInput")

    out_e = nc.dram_tensor("out", [S, D], fp32, kind="ExternalOutput")

    with tile.TileContext(nc) as tc:
        # ------------- persistent pools -------------
        const_cm = tc.tile_pool(name="const", bufs=1)
        const = const_cm.__enter__()
        dram_cm = tc.tile_pool(name="dstage", bufs=1, space="DRAM")
        dram = dram_cm.__enter__()
        bstage = dram.tile([S, H, S], bf16)
        resid_cm = tc.tile_pool(name="resid", bufs=1)  # ctxT/xln (ph3-7)
        resid = resid_cm.__enter__()
        bigE_cm = tc.tile_pool(name="bigE", bufs=1)   # Qt/Kt/V (ph1-3)
        bigE = bigE_cm.__enter__()

        ident_b = const.tile([P, P], bf16)
        make_identity(nc, ident_b)
        ident_f = const.tile([P, P], fp32)
        make_identity(nc, ident_f)

        eps_log_c = const.tile([P, 1], fp32)
        nc.vector.memset(eps_log_c, EPS_LOG)
        eps_ln_c = const.tile([P, 1], fp32)
        nc.vector.memset(eps_ln_c, EPS_LN)

        c1A = const.tile([P, P], bf16)
        c1B = const.tile([P, P], bf16)
        c2A = const.tile([P, P], bf16)
        c2B = const.tile([P, P], bf16)
        nc.scalar.dma_start(out=c1A, in_=c1A_e[:, :])
        nc.scalar.dma_start(out=c1B, in_=c1B_e[:, :])
        nc.scalar.dma_start(out=c2A, in_=c2A_e[:, :])
        nc.scalar.dma_start(out=c2B, in_=c2B_e[:, :])
        bc1A = const.tile([P, 1], fp32)
        bc1B = const.tile([P, 1], fp32)
        bc2c = const.tile([P, 1], fp32)
        nc.scalar.dma_start(out=bc1A, in_=bc1A_e[:, :])
        nc.scalar.dma_start(out=bc1B, in_=bc1B_e[:, :])
        nc.scalar.dma_start(out=bc2c, in_=bc2c_e[:, :])
        bqc = const.tile([P, ND], fp32)
        bkc = const.tile([P, ND], fp32)
        bf1c = const.tile([P, NFF], fp32)
        nc.scalar.dma_start(out=bqc, in_=bqc_e[:, :])
        nc.scalar.dma_start(out=bkc, in_=bkc_e[:, :])
        nc.scalar.dma_start(out=bf1c, in_=bf1c_e[:, :])
        bvr = const.tile([1, D], bf16)
        bmr = const.tile([1, D], bf16)
        bf2r = const.tile([1, D], bf16)
        onesb = const.tile([1, P], bf16)
        nc.scalar.dma_start(out=bvr, in_=bvr_e[:, :])
        nc.scalar.dma_start(out=bmr, in_=bmr_e[:, :])
        nc.scalar.dma_start(out=bf2r, in_=bf2r_e[:, :])
        nc.scalar.dma_start(out=onesb, in_=onesb_e[:, :])

        # LN param broadcast tiles [128, D]
        g1b = const.tile([P, D], bf16)
        b1b = const.tile([P, D], bf16)
        g2b = const.tile([P, D], bf16)
        b2b = const.tile([P, D], bf16)
        for dst, src_e in ((g1b, g1r_e), (b1b, b1r_e), (g2b, g2r_e), (b2b, b2r_e)):
            row = const.tile([1, D], bf16, tag="lnrow", name="lnrow")
            nc.scalar.dma_start(out=row, in_=src_e[:, :])
            nc.gpsimd.partition_broadcast(dst, row[0:1, :])

        # attention-phase residents
        QtT = bigE.tile([P, ND, S], bf16)      # [o-part, o-blk, s]  ((Wq x + bq)/8)
        KtT = bigE.tile([P, ND, S], bf16)
        Vsb = bigE.tile([P, NQT, D], bf16)     # [k-part, k-blk, o]
        ctxT = resid.tile([P, ND, S], bf16)    # [(h,dh)-part, blk, q]
        xln = resid.tile([P, NQT, D], fp32)    # LN1 out [s-part, s-blk, d]

        # =========== Phase 1: projections ===========
        with (
            tc.tile_pool(name="p1x", bufs=1) as p1x,
            tc.tile_pool(name="p1w", bufs=3) as p1w,
            tc.tile_pool(name="p1ps", bufs=1, space="PSUM") as p1ps,
        ):
            qT = p1x.tile([P, ND, S], bf16)
            kvT = p1x.tile([P, ND, S], bf16)
            for dblk in range(ND):
                nc.sync.dma_start(
                    out=qT[:, dblk, :],
                    in_=qT_e[dblk * P : (dblk + 1) * P, :],
                )
                nc.sync.dma_start(
                    out=kvT[:, dblk, :],
                    in_=kvT_e[dblk * P : (dblk + 1) * P, :],
                )

            for wsrc, xsb, dst, bcol in (
                (wqT_e, qT, QtT, bqc),
                (wkT_e, kvT, KtT, bkc),
            ):
                psums = [p1ps.tile([P, S], fp32, tag=f"pp{i}", name=f"pp{i}") for i in range(ND)]
                for dblk in range(ND):
                    wch = p1w.tile([P, D], bf16, tag="wch")
                    nc.sync.dma_start(
                        out=wch, in_=wsrc[dblk * P : (dblk + 1) * P, :]
                    )
                    for ob in range(ND):
                        nc.tensor.matmul(
                            psums[ob],
                            wch[:, ts(ob, P)],
                            xsb[:, dblk, :],
                            start=(dblk == 0),
                            stop=(dblk == ND - 1),
                        )
                for ob in range(ND):
                    nc.vector.tensor_scalar_add(
                        dst[:, ob, :], psums[ob], bcol[:, ob : ob + 1]
                    )

            vps = [
                [p1ps.tile([P, S], fp32, tag=f"pp{st * 2 + oh}", name=f"vp{st}{oh}") for oh in range(2)]
                for st in range(NQT)
            ]
            for st in range(NQT):
                for oh in range(2):
                    nc.tensor.matmul(
                        vps[st][oh], onesb, bvr[:, ts(oh, S)],
                        start=True, stop=False,
                    )
            for dblk in range(ND):
                wch = p1w.tile([P, D], bf16, tag="wch")
                nc.sync.dma_start(out=wch, in_=wvT_e[dblk * P : (dblk + 1) * P, :])
                for st in range(NQT):
                    for oh in range(2):
                        nc.tensor.matmul(
                            vps[st][oh],
                            kvT[:, dblk, ts(st, P)],
                            wch[:, ts(oh, S)],
                            start=False,
                            stop=(dblk == ND - 1),
                        )
            for st in range(NQT):
                for oh in range(2):
                    nc.vector.tensor_copy(Vsb[:, st, ts(oh, S)], vps[st][oh])

        # =========== Phase 2+3: conv bias + attention, per qtile ===========
        with (
            tc.tile_pool(name="pbias", bufs=2) as pbias,
            tc.tile_pool(name="pcv", bufs=2) as pcv,
            tc.tile_pool(name="pcvps", bufs=1, space="PSUM") as pcvps,
            tc.tile_pool(name="pc2ps", bufs=1, space="PSUM") as pc2ps,
            tc.tile_pool(name="p3sb", bufs=2) as p3sb,
            tc.tile_pool(name="p3ps", bufs=1, space="PSUM") as p3ps,
        ):
            prev_last_exp = None
            for blk in range(NQT // 2):
                # ---- conv bias for qtiles 2*blk, 2*blk+1 ----
                logms = {}
                first_log = None
                last_gelu = None
                for sub in range(2):
                    qt = blk * 2 + sub
                    for hemi in range(2):
                        qbase = qt * P + hemi * 64
                        amt = pcv.tile([P, NQI, S], fp32, tag="amt")
                        for g in range(8):
                            src = bass.AP(
                                tensor=amap_e,
                                offset=(1 + qbase + NQI * g) * AM + 1,
                                ap=[[AM * AM, CH], [AM, NQI], [1, S]],
                            )
                            nc.sync.dma_start(out=amt[CH * g : CH * (g + 1)], in_=src)
                        logm = pcv.tile([P, NQI * S], bf16, tag="logm", bufs=4)
                        li = nc.scalar.activation(
                            logm, amt.rearrange("p a b -> p (a b)"), AF.Ln,
                            bias=eps_log_c, scale=1.0,
                        )
                        if first_log is None:
                            first_log = li
                        logms[(sub, hemi)] = logm
                # keep the ACT table from ping-ponging between the Ln/Exp set
                # and the Gelu set: this block's logs come after the previous
                # block's last softmax exp
                if prev_last_exp is not None and first_log is not None:
                    add_dep_helper(first_log.ins, prev_last_exp.ins, False,
                                   "act-table grouping")
                for sub in range(2):
                    qt = blk * 2 + sub
                    for hemi in range(2):
                        qbase = qt * P + hemi * 64
                        logm = logms[(sub, hemi)]
                        c2sb = pcv.tile([P, NQI, S], bf16, tag="c2sb", bufs=2)
                        for ch in range(NQI):
                            pAB = pcvps.tile([P, 2, S], fp32, tag="pAB", bufs=2)
                            nc.tensor.matmul(
                                pAB[:, 0, :], c1A, logm[:, ts(ch, S)],
                                start=True, stop=True,
                            )
                            nc.tensor.matmul(
                                pAB[:, 1, :], c1B, logm[:, ts(ch, S)],
                                start=True, stop=True,
                            )
                            gAB = pcv.tile([P, 2, S], bf16, tag="gAB", bufs=2)
                            nc.scalar.activation(
                                gAB[:, 0, :], pAB[:, 0, :], AF.Gelu,
                                bias=bc1A, scale=1.0,
                            )
                            last_gelu = nc.scalar.activation(
                                gAB[:, 1, :], pAB[:, 1, :], AF.Gelu,
                                bias=bc1B, scale=1.0,
                            )
                            pC = pc2ps.tile([P, S], fp32, tag="pC")
                            nc.tensor.matmul(
                                pC, c2A, gAB[:, 0, :], start=True, stop=False
                            )
                            nc.tensor.matmul(
                                pC, c2B, gAB[:, 1, :], start=False, stop=True
                            )
                            nc.vector.tensor_scalar_add(c2sb[:, ch, :], pC, bc2c)
                        # scatter to DRAM bias stage in [q, h, k] order
                        for g in range(8):
                            q0 = qbase + NQI * g
                            nc.gpsimd.dma_start(
                                out=bstage[q0 : q0 + NQI].rearrange("i h k -> h i k"),
                                in_=c2sb[CH * g : CH * (g + 1)],
                            )

                exp_insts = []
                for sub in range(2):
                    qt = blk * 2 + sub
                    biasq = pbias.tile([P, H, S], bf16, tag="biasq")
                    nc.sync.dma_start(
                        out=biasq.rearrange("p h k -> p (h k)"),
                        in_=bstage[qt * P : (qt + 1) * P].rearrange("q h k -> q (h k)"),
                    )
                    _attention_qt(nc, qt, biasq, QtT, KtT, Vsb, ctxT,
                                  ident_b, p3sb, p3ps, pcvps, exp_insts)
                # this block's exps come after this block's last gelu
                if last_gelu is not None and exp_insts:
                    add_dep_helper(exp_insts[0].ins, last_gelu.ins, False,
                                   "act-table grouping")
                prev_last_exp = exp_insts[-1] if exp_insts else None

        bigE_cm.__exit__(None, None, None)

        # =========== Phase 4: merge + residual + LN1 ===========
        with (
            tc.tile_pool(name="p4sb", bufs=3) as p4sb,
            tc.tile_pool(name="p4ps", bufs=1, space="PSUM") as p4ps,
        ):
            wmr = p4sb.tile([P, ND, D], bf16, tag="wmr", bufs=1)
            for dblk in range(ND):
                nc.sync.dma_start(
                    out=wmr[:, dblk, :], in_=wmT_e[dblk * P : (dblk + 1) * P, :]
                )
            for st in range(NQT):
                mps = p4ps.tile([P, 2, S], fp32, tag="mps", bufs=2)
                for oh in range(2):
                    nc.tensor.matmul(
                        mps[:, oh, :], onesb, bmr[:, ts(oh, S)],
                        start=True, stop=False,
                    )
                for dblk in range(ND):
                    for oh in range(2):
                        nc.tensor.matmul(
                            mps[:, oh, :],
                            ctxT[:, dblk, ts(st, P)],
                            wmr[:, dblk, ts(oh, S)],
                            start=False,
                            stop=(dblk == ND - 1),
                        )
                qtile = p4sb.tile([P, D], fp32, tag="qtile")
                nc.sync.dma_start(out=qtile, in_=qin_e[st * P : (st + 1) * P, :])
                x1 = p4sb.tile([P, D], fp32, tag="x1")
                for oh in range(2):
                    nc.vector.tensor_tensor(
                        out=x1[:, ts(oh, S)], in0=mps[:, oh, :],
                        in1=qtile[:, ts(oh, S)], op=ALU.add,
                    )
                _layernorm(nc, p4sb, xln[:, st, :], x1, g1b, b1b, eps_ln_c)

        bigL_cm = tc.tile_pool(name="bigL", bufs=1)
        bigL = bigL_cm.__enter__()
        fdt_ = fp8 if FFN_FP8 else bf16
        xlnT = bigL.tile([P, ND, S], fdt_)
        y1T = bigL.tile([P, NFF, S], fdt_)
        wf2r = bigL.tile([P, NFF // 2, 2 * D], fdt_)
        for fb in range(NFF // 2):
            nc.sync.dma_start(out=wf2r[:, fb, :], in_=wf2P_e[fb])

        # =========== Phase 5: transpose x_ln ===========
        with tc.tile_pool(name="p5ps", bufs=2, space="PSUM") as p5ps:
            for dblk in range(ND):
                tp = p5ps.tile([P, S], fp32, tag="tp")
                for st in range(NQT):
                    nc.tensor.transpose(
                        tp[:, ts(st, P)], xln[:, st, ts(dblk, P)], ident_f
                    )
                nc.vector.tensor_copy(xlnT[:, dblk, :], tp)

        bigL_cm = tc.tile_pool(name="bigL", bufs=1)
        bigL = bigL_cm.__enter__()
        fdt_ = fp8 if FFN_FP8 else bf16
        xlnT = bigL.tile([P, ND, S], bf16)
        y1T = bigL.tile([P, NFF, S], fdt_)
        wf2r = bigL.tile([P, NFF // 2, 2 * D], fdt_)
        for fb in range(NFF // 2):
            nc.sync.dma_start(out=wf2r[:, fb, :], in_=wf2P_e[fb])

        # =========== Phase 5: transpose x_ln ===========
        with tc.tile_pool(name="p5ps", bufs=2, space="PSUM") as p5ps:
            for dblk in range(ND):
                tp = p5ps.tile([P, S], fp32, tag="tp")
                for st in range(NQT):
                    nc.tensor.transpose(
                        tp[:, ts(st, P)], xln[:, st, ts(dblk, P)], ident_f
                    )
                nc.vector.tensor_copy(xlnT[:, dblk, :], tp)

        # =========== Phase 6: FFN1 + relu ===========
        with (
            tc.tile_pool(name="p6w", bufs=4) as p6w,
            tc.tile_pool(name="p6ps", bufs=2, space="PSUM") as p6ps,
        ):
            for ffb in range(NFF):
                fps = p6ps.tile([P, S], fp32, tag="fps")
                if FFN_FP8:
                    wf1c = p6w.tile([P, ND // 2, 2, P], fp8, tag="wf1c")
                    nc.sync.dma_start(
                        out=wf1c.rearrange("p a b c -> p (a b c)"), in_=wf1P_e[ffb]
                    )
                    for b in range(ND // 2):
                        nc.tensor.matmul(
                            fps,
                            wf1c[:, b, :, :],
                            xlnT[:, 2 * b : 2 * b + 2, :],
                            start=(b == 0), stop=(b == ND // 2 - 1),
                            perf_mode=DR,
                        )
                else:
                    wf1c = p6w.tile([P, ND, P], bf16, tag="wf1c")
                    nc.sync.dma_start(
                        out=wf1c.rearrange("p a b -> p (a b)"), in_=wf1P_e[ffb]
                    )
                    for dblk in range(ND):
                        nc.tensor.matmul(
                            fps,
                            wf1c[:, dblk, :],
                            xlnT[:, dblk, :],
                            start=(dblk == 0), stop=(dblk == ND - 1),
                        )
                nc.vector.tensor_scalar(
                    out=y1T[:, ffb, :], in0=fps,
                    scalar1=bf1c[:, ffb : ffb + 1], scalar2=0.0,
                    op0=ALU.add, op1=ALU.max,
                )

        # =========== Phase 7: FFN2 + residual + LN2 + out ===========
        with (
            tc.tile_pool(name="p7sb", bufs=2) as p7sb,
            tc.tile_pool(name="p7ps", bufs=1, space="PSUM") as p7ps,
        ):
            for st in range(NQT):
                fps2 = p7ps.tile([P, 2, S], fp32, tag="fps2", bufs=2)
                for oh in range(2):
                    nc.tensor.matmul(
                        fps2[:, oh, :], onesb, bf2r[:, ts(oh, S)],
                        start=True, stop=False,
                    )
                for ffb in range(NFF):
                    wch = wf2r[:, ffb // 2, (ffb % 2) * D : (ffb % 2 + 1) * D]
                    for oh in range(2):
                        nc.tensor.matmul(
                            fps2[:, oh, :],
                            y1T[:, ffb, ts(st, P)],
                            wch[:, ts(oh, S)],
                            start=False,
                            stop=(ffb == NFF - 1),
                        )
                x2 = p7sb.tile([P, D], fp32, tag="x2")
                for oh in range(2):
                    nc.vector.tensor_tensor(
                        out=x2[:, ts(oh, S)], in0=fps2[:, oh, :],
                        in1=xln[:, st, ts(oh, S)], op=ALU.add,
                    )
                xout = p7sb.tile([P, D], fp32, tag="xout")
                _layernorm(nc, p7sb, xout, x2, g2b, b2b, eps_ln_c)
                nc.sync.dma_start(out=out_e[st * P : (st + 1) * P, :], in_=xout)

        bigL_cm.__exit__(None, None, None)
        resid_cm.__exit__(None, None, None)
        dram_cm.__exit__(None, None, None)
        const_cm.__exit__(None, None, None)

    nc.finalize()
    return nc


def _prep_inputs(q, kv, attn_map, Wq, bq, Wk, bk, Wv, bv, Wm, bm,
                 Wc1, bc1, Wc2, bc2, Wf1, bf1, Wf2, bf2, g1, b1, g2, b2):
    """Host-side packing. Returns (shared dict, per-core list of dicts)."""
    f32 = np.float32
    bf = ml_dtypes.bfloat16

    def c(a):
        return np.ascontiguousarray(np.asarray(a), dtype=f32)

    def cb(a):
        return np.ascontiguousarray(np.asarray(a, dtype=f32)).astype(bf)

    Wq, Wk, Wv, Wm = c(Wq), c(Wk), c(Wv), c(Wm)
    Wc1, Wc2, Wf1, Wf2 = c(Wc1), c(Wc2), c(Wf1), c(Wf2)
    bq, bk, bv, bm = c(bq), c(bk), c(bv), c(bm)
    bc1, bc2, bf1, bf2 = c(bc1), c(bc2), c(bf1), c(bf2)
    g1, b1, g2, b2 = c(g1), c(b1), c(g2), c(b2)

    fdt = ml_dtypes.float8_e4m3 if FFN_FP8 else bf

    # FFN1 weights packed so each SBUF partition gets a contiguous row.
    # fp8/DoubleRow: wf1P[ffb, p, ((b,j),f)] = Wf1.T[(2b+j)*128+p, ffb*128+f]
    # bf16:          wf1P[ffb, p, (n,f)]     = Wf1.T[n*128+p, ffb*128+f]
    Wf1T = np.ascontiguousarray(Wf1.T)            # [D, FF]
    wf1P = np.ascontiguousarray(
        Wf1T.reshape(ND, P, NFF, P).transpose(2, 1, 0, 3).reshape(NFF, P, D)
    ).astype(fdt)
    # FFN2 weights: wf2P[fb, p, (j,d)] = Wf2.T[(2fb+j)*128+p, d]
    Wf2T = np.ascontiguousarray(Wf2.T)            # [FF, D]
    wf2P = np.ascontiguousarray(
        Wf2T.reshape(NFF // 2, 2, P, D).transpose(0, 2, 1, 3).reshape(NFF // 2, P, 2 * D)
    ).astype(fdt)

    shared = {
        "wqT": cb(Wq.T * 0.125), "wkT": cb(Wk.T), "wvT": cb(Wv.T),
        "wmT": cb(Wm.T),
        "wf1P": wf1P,
        "wf2P": wf2P,
        "bqc": c((bq / 8.0).reshape(ND, P).T),
        "bkc": c(bk.reshape(ND, P).T),
        "bf1c": c(bf1.reshape(NFF, P).T),
        "bvr": cb(bv.reshape(1, D)), "bmr": cb(bm.reshape(1, D)),
        "bf2r": cb(bf2.reshape(1, D)),
        "onesb": np.ones((1, P), bf),
        "g1r": cb(g1.reshape(1, D)), "b1r": cb(b1.reshape(1, D)),
        "g2r": cb(g2.reshape(1, D)), "b2r": cb(b2.reshape(1, D)),
    }
    # conv block-diag lhsT [K, M]: out[(g,oh)] = sum_c lhsT[(g,c),(g,oh)] rhs[(g,c)]
    c1A = np.zeros((P, P), f32)
    c1B = np.zeros((P, P), f32)
    c2A = np.zeros((P, P), f32)
    c2B = np.zeros((P, P), f32)
    for g in range(8):
        sl = slice(g * 16, g * 16 + 16)
        c1A[sl, sl] = Wc1[0:16, :].T     # [c, oh]
        c1B[sl, sl] = Wc1[16:32, :].T
        c2A[sl, sl] = Wc2[:, 0:16].T     # [ci, h]
        c2B[sl, sl] = Wc2[:, 16:32].T
    shared["c1A"] = c1A.astype(bf)
    shared["c1B"] = c1B.astype(bf)
    shared["c2A"] = c2A.astype(bf)
    shared["c2B"] = c2B.astype(bf)
    shared["bc1A"] = np.tile(bc1[0:16], 8).reshape(P, 1).astype(f32)
    shared["bc1B"] = np.tile(bc1[16:32], 8).reshape(P, 1).astype(f32)
    shared["bc2c"] = np.tile(bc2, 8).reshape(P, 1).astype(f32)

    q = c(q)
    kv = c(kv)
    attn_map = np.asarray(attn_map)
    per_core = []
    for b in range(B):
        per_core.append({
            "qT": np.ascontiguousarray(q[b].T).astype(bf),
            "kvT": np.ascontiguousarray(kv[b].T).astype(bf),
            "qin": q[b],
            "amap": c(attn_map[b]),
        })
    return shared, per_core


def kernel(**inputs):
    if "nc" not in _CACHED:
        _CACHED["nc"] = build_program()
    nc = _CACHED["nc"]
    shared, per_core = _prep_inputs(**inputs)
    in_maps = [dict(shared, **pc) for pc in per_core]
    res = run_bass_kernel_spmd(nc, in_maps, list(range(B)))
    out = np.stack([res.results[i]["out"] for i in range(B)], axis=0)
    return out.astype(np.float32)
